# revision 4
# baseline (speedup 1.0000x reference)
"""Causal multi-head attention on 8 TRN2 NeuronCores.

Problem: x[4,2048,768], 12 heads x 64 dim, causal softmax attention.
Sharding: TP2 x DP4 -- core c handles batch c//2 and heads (c%2)*6..+6.
Each core computes a partial output (sum over its 6 heads); the host sums
the two partials per batch and adds b_O.

All matmuls run in bf16 (fp32 PSUM accumulation). The 1/sqrt(d_head)
scale and the Q/K/V biases are folded into the weights host-side (biases
enter through an augmented all-ones contraction row of x^T).
"""

import numpy as np
import ml_dtypes

import concourse.bacc as bacc
import concourse.tile as tile
from concourse import mybir
from concourse.bass_utils import run_bass_kernel_spmd
from concourse.masks import make_identity

BF16 = ml_dtypes.bfloat16

P = 128          # partitions
S = 2048         # sequence length
DM = 768         # d_model
DH = 64          # d_head
HPC = 6          # heads per core
NPAIR = HPC // 2
MC = 7           # m-chunks of x^T (768 rows + 1 bias row, padded to 896)
NKT = S // P     # key tiles (16)
VSLOT = DH + 1   # per-(ktile, head) V slot width: 64 data + ones column
NCORES = 8

# PT row offsets: row ki holds S^T[k in ki-tile, q in [ki*128, S)]
PT_OFF = [0] * (NKT + 1)
for _ki in range(NKT):
    PT_OFF[_ki + 1] = PT_OFF[_ki] + (S - _ki * P)
PT_W = PT_OFF[NKT]  # 17408


def build(nc):
    bf = mybir.dt.bfloat16
    f32 = mybir.dt.float32
    EXP = mybir.ActivationFunctionType.Exp
    IDENT = mybir.ActivationFunctionType.Identity

    xT_d = nc.dram_tensor("xT", [P, MC * S], bf, kind="ExternalInput")
    wq_d = nc.dram_tensor("wq", [P, NPAIR * MC * P], bf, kind="ExternalInput")
    wk_d = nc.dram_tensor("wk", [P, NPAIR * MC * P], bf, kind="ExternalInput")
    wv_d = nc.dram_tensor("wv", [P, MC * HPC * DH], bf, kind="ExternalInput")
    wo_d = nc.dram_tensor("wo", [P, NPAIR * DM], bf, kind="ExternalInput")
    out_d = nc.dram_tensor("out", [S, DM], bf, kind="ExternalOutput")

    from contextlib import ExitStack
    with tile.TileContext(nc) as tc, ExitStack() as ctx:
        const = ctx.enter_context(tc.tile_pool(name="const", bufs=1))
        work = ctx.enter_context(tc.tile_pool(name="work", bufs=3))
        st_pool = ctx.enter_context(tc.tile_pool(name="st", bufs=2, space="PSUM"))
        z_pool = ctx.enter_context(tc.tile_pool(name="zp", bufs=2, space="PSUM"))
        op_pool = ctx.enter_context(tc.tile_pool(name="opp", bufs=1, space="PSUM"))

        # ---- constants / inputs to SBUF ----
        xT_sb = const.tile([P, MC * S], bf)
        # column-group DMAs so the first q-group's matmuls start early
        for g in range(4):
            nc.sync.dma_start(
                out=xT_sb.rearrange("p (c s) -> p c s", s=S)[:, :, g * 512:(g + 1) * 512],
                in_=xT_d.rearrange("p (c s) -> p c s", s=S)[:, :, g * 512:(g + 1) * 512],
            )
        wq_sb = const.tile([P, NPAIR * MC * P], bf)
        nc.sync.dma_start(out=wq_sb, in_=wq_d[:])
        wk_sb = const.tile([P, NPAIR * MC * P], bf)
        nc.sync.dma_start(out=wk_sb, in_=wk_d[:])
        wv_sb = const.tile([P, MC * HPC * DH], bf)
        nc.sync.dma_start(out=wv_sb, in_=wv_d[:])
        wo_sb = const.tile([P, NPAIR * DM], bf)
        nc.sync.dma_start(out=wo_sb, in_=wo_d[:])

        ident = const.tile([P, P], bf)
        make_identity(nc, ident)
        # causal keep-mask in [k, q] layout: 1 where k <= q else 0
        cmask = const.tile([P, P], bf)
        nc.gpsimd.memset(cmask, 1.0)
        nc.gpsimd.affine_select(
            out=cmask, in_=cmask,
            compare_op=mybir.AluOpType.is_ge,
            fill=0.0, base=0,
            pattern=[[1, P]],       # iota = q - k ; keep when >= 0
            channel_multiplier=-1,
        )

        qt_sb = const.tile([P, NPAIR * S], bf)   # Q^T per pair [2*64, S]
        kt_sb = const.tile([P, NPAIR * S], bf)
        vsb = const.tile([P, NKT * HPC * VSLOT], bf)
        nc.vector.memset(vsb, 1.0)               # ones survive in col 64 of each slot
        Zst = const.tile([P, NKT * HPC * DH], bf)
        PT = const.tile([P, PT_W], bf)

        # ---- phase 1: projections ----
        # Q^T / K^T, head-pair packed: psum[128(d of 2 heads), 512(q)]
        for g in range(4):
            for p in range(NPAIR):
                for wsb, dst in ((wq_sb, qt_sb), (wk_sb, kt_sb)):
                    ps = st_pool.tile([P, 1024], f32, tag="st")
                    for mc in range(MC):
                        nc.tensor.matmul(
                            ps[:, 0:512],
                            lhsT=wsb[:, (p * MC + mc) * P:(p * MC + mc + 1) * P],
                            rhs=xT_sb[:, mc * S + g * 512: mc * S + g * 512 + 512],
                            start=(mc == 0), stop=(mc == MC - 1),
                        )
                    nc.vector.tensor_copy(
                        dst[:, p * S + g * 512: p * S + g * 512 + 512], ps[:, 0:512]
                    )
        # V in [k, d] layout, all 6 heads side by side; strided copy into 65-wide slots
        for kt in range(NKT):
            ps = st_pool.tile([P, 1024], f32, tag="st")
            for mc in range(MC):
                nc.tensor.matmul(
                    ps[:, 0:HPC * DH],
                    lhsT=xT_sb[:, mc * S + kt * P: mc * S + (kt + 1) * P],
                    rhs=wv_sb[:, mc * HPC * DH:(mc + 1) * HPC * DH],
                    start=(mc == 0), stop=(mc == MC - 1),
                )
            dst = vsb[:, kt * HPC * VSLOT:(kt + 1) * HPC * VSLOT]
            dst = dst.rearrange("p (h w) -> p h w", w=VSLOT)[:, :, 0:DH]
            src = ps[:, 0:HPC * DH].rearrange("p (h d) -> p h d", d=DH)
            nc.vector.tensor_copy(dst, src)

        # ---- phase 2: attention per head ----
        for h in range(HPC):
            pr, half = divmod(h, 2)
            po = DH * half                     # partition offset of this head in the pair
            qo = pr * S
            for ki in range(NKT):
                cols = S - ki * P
                kt_slice = kt_sb[po:po + DH, qo + ki * P: qo + (ki + 1) * P]
                c0 = 0
                while c0 < cols:
                    w = min(1024, cols - c0)
                    ps = st_pool.tile([P, 1024], f32, tag="st")
                    for s0 in range(0, w, 512):
                        sw = min(512, w - s0)
                        nc.tensor.matmul(
                            ps[:, s0:s0 + sw],
                            lhsT=kt_slice,
                            rhs=qt_sb[po:po + DH,
                                      qo + ki * P + c0 + s0: qo + ki * P + c0 + s0 + sw],
                            start=True, stop=True,
                        )
                    nc.scalar.activation(
                        out=PT[:, PT_OFF[ki] + c0: PT_OFF[ki] + c0 + w],
                        in_=ps[:, 0:w], func=EXP,
                    )
                    c0 += w
                # causal mask on the diagonal block
                nc.vector.tensor_mul(
                    PT[:, PT_OFF[ki]:PT_OFF[ki] + P],
                    PT[:, PT_OFF[ki]:PT_OFF[ki] + P],
                    cmask,
                )
                # PV for q-tile qt == ki (all needed PT rows are ready)
                qt = ki
                zt = z_pool.tile([P, 512], f32, tag="z")
                for k2 in range(qt + 1):
                    nc.tensor.matmul(
                        zt[:, 0:VSLOT],
                        lhsT=PT[:, PT_OFF[k2] + (qt - k2) * P: PT_OFF[k2] + (qt - k2 + 1) * P],
                        rhs=vsb[:, (k2 * HPC + h) * VSLOT:(k2 * HPC + h + 1) * VSLOT],
                        start=(k2 == 0), stop=(k2 == qt),
                    )
                r = work.tile([P, 1], f32, tag="r")
                nc.vector.reciprocal(r, zt[:, DH:DH + 1])
                nc.scalar.activation(
                    out=Zst[:, (qt * HPC + h) * DH:(qt * HPC + h + 1) * DH],
                    in_=zt[:, 0:DH], func=IDENT, scale=r[:, 0:1],
                )

        # ---- phase 3: Z transpose + output projection ----
        for qt in range(NKT):
            zts = work.tile([P, NPAIR * P], bf, tag="zt")
            for c in range(NPAIR):
                trp = z_pool.tile([P, P], bf, tag="z")
                nc.tensor.transpose(
                    trp[:, 0:P],
                    Zst[:, qt * HPC * DH + c * P: qt * HPC * DH + (c + 1) * P],
                    ident,
                )
                nc.vector.tensor_copy(zts[:, c * P:(c + 1) * P], trp[:, 0:P])
            op = op_pool.tile([P, DM], f32, tag="op")
            for n0, nw in ((0, 512), (512, 256)):
                for c in range(NPAIR):
                    nc.tensor.matmul(
                        op[:, n0:n0 + nw],
                        lhsT=zts[:, c * P:(c + 1) * P],
                        rhs=wo_sb[:, c * DM + n0: c * DM + n0 + nw],
                        start=(c == 0), stop=(c == NPAIR - 1),
                    )
            osb = work.tile([P, DM], bf, tag="o")
            nc.vector.tensor_copy(osb, op)
            nc.sync.dma_start(out=out_d[qt * P:(qt + 1) * P, :], in_=osb)

    nc.compile()
    return nc


_CACHED_NC = None


def _get_nc():
    global _CACHED_NC
    if _CACHED_NC is None:
        nc = bacc.Bacc("TRN2", target_bir_lowering=False, debug=False,
                       num_devices=NCORES)
        _CACHED_NC = build(nc)
    return _CACHED_NC


def _prep_core_inputs(x, W_Q, W_K, W_V, W_O, b_Q, b_K, b_V):
    """Host-side shard prep for one (batch, head-group) core.

    x: [S, DM] f32; W_*: [6, DM, DH] (W_O: [6, DH, DM]); b_*: [6, DH].
    Returns dict of bf16 SBUF-image arrays.
    """
    scale = 1.0 / np.sqrt(np.float32(DH))

    xT_aug = np.zeros((MC * P, S), np.float32)
    xT_aug[:DM] = x.T
    xT_aug[DM] = 1.0                      # bias row

    def pack_pairs(W, b):                 # -> [P, NPAIR*MC*P]
        img = np.zeros((P, NPAIR * MC * P), np.float32)
        for p in range(NPAIR):
            aug = np.zeros((MC * P, P), np.float32)
            aug[:DM, 0:DH] = W[2 * p]
            aug[:DM, DH:2 * DH] = W[2 * p + 1]
            aug[DM, 0:DH] = b[2 * p]
            aug[DM, DH:2 * DH] = b[2 * p + 1]
            for mc in range(MC):
                img[:, (p * MC + mc) * P:(p * MC + mc + 1) * P] = aug[mc * P:(mc + 1) * P]
        return img

    wq_img = pack_pairs(W_Q * scale, b_Q * scale)
    wk_img = pack_pairs(W_K, b_K)

    wv_aug = np.zeros((MC * P, HPC * DH), np.float32)
    wv_aug[:DM] = np.concatenate([W_V[h] for h in range(HPC)], axis=1)
    wv_aug[DM] = b_V.reshape(-1)
    wv_img = np.zeros((P, MC * HPC * DH), np.float32)
    for mc in range(MC):
        wv_img[:, mc * HPC * DH:(mc + 1) * HPC * DH] = wv_aug[mc * P:(mc + 1) * P]

    wo_flat = np.concatenate([W_O[h] for h in range(HPC)], axis=0)  # [384, DM]
    wo_img = np.zeros((P, NPAIR * DM), np.float32)
    for c in range(NPAIR):
        wo_img[:, c * DM:(c + 1) * DM] = wo_flat[c * P:(c + 1) * P]

    return {
        "xT": xT_aug.reshape(MC, P, S).transpose(1, 0, 2).reshape(P, MC * S).astype(BF16),
        "wq": wq_img.astype(BF16),
        "wk": wk_img.astype(BF16),
        "wv": wv_img.astype(BF16),
        "wo": wo_img.astype(BF16),
    }


def kernel(normalized_resid_pre, W_Q, W_K, W_V, W_O, b_Q, b_K, b_V, b_O):
    x = np.asarray(normalized_resid_pre, np.float32)
    nc = _get_nc()

    in_maps = []
    for core in range(NCORES):
        b, t = divmod(core, 2)
        hs = slice(t * HPC, (t + 1) * HPC)
        in_maps.append(_prep_core_inputs(
            x[b], np.asarray(W_Q)[hs], np.asarray(W_K)[hs], np.asarray(W_V)[hs],
            np.asarray(W_O)[hs], np.asarray(b_Q)[hs], np.asarray(b_K)[hs],
            np.asarray(b_V)[hs],
        ))

    res = run_bass_kernel_spmd(nc, in_maps, core_ids=list(range(NCORES)))
    out = np.zeros((4, S, DM), np.float32)
    for b in range(4):
        out[b] = (res.results[2 * b]["out"].astype(np.float32)
                  + res.results[2 * b + 1]["out"].astype(np.float32))
        out[b] += np.asarray(b_O, np.float32)
    return out


# revision 7
# speedup vs baseline: 1.1589x; 1.1589x over previous
"""Causal multi-head attention on 8 TRN2 NeuronCores.

Problem: x[4,2048,768], 12 heads x 64 dim, causal softmax attention.
Sharding: TP2 x DP4 -- core c handles batch c//2 and heads (c%2)*6..+6.
Each core computes a partial output (sum over its 6 heads); the host sums
the two partials per batch and adds b_O.

All matmuls run in bf16 (fp32 PSUM accumulation). The 1/sqrt(d_head)
scale and the Q/K/V biases are folded into the weights host-side (biases
enter through an augmented all-ones contraction row of x^T).
"""

import numpy as np
import ml_dtypes

import concourse.bacc as bacc
import concourse.tile as tile
from concourse import mybir
from concourse.bass_utils import run_bass_kernel_spmd
from concourse.masks import make_identity

BF16 = ml_dtypes.bfloat16

P = 128          # partitions
S = 2048         # sequence length
DM = 768         # d_model
DH = 64          # d_head
HPC = 6          # heads per core
NPAIR = HPC // 2
MC = 7           # m-chunks of x^T (768 rows + 1 bias row, padded to 896)
NKT = S // P     # key tiles (16)
VSLOT = DH + 1   # per-(ktile, head) V slot width: 64 data + ones column
NCORES = 8

# PT row offsets: row ki holds S^T[k in ki-tile, q in [ki*128, S)]
PT_OFF = [0] * (NKT + 1)
for _ki in range(NKT):
    PT_OFF[_ki + 1] = PT_OFF[_ki] + (S - _ki * P)
PT_W = PT_OFF[NKT]  # 17408


def build(nc):
    bf = mybir.dt.bfloat16
    f32 = mybir.dt.float32
    EXP = mybir.ActivationFunctionType.Exp
    IDENT = mybir.ActivationFunctionType.Identity

    xT_d = nc.dram_tensor("xT", [P, MC * S], bf, kind="ExternalInput")
    wq_d = nc.dram_tensor("wq", [P, NPAIR * MC * P], bf, kind="ExternalInput")
    wk_d = nc.dram_tensor("wk", [P, NPAIR * MC * P], bf, kind="ExternalInput")
    wv_d = nc.dram_tensor("wv", [P, MC * HPC * DH], bf, kind="ExternalInput")
    wo_d = nc.dram_tensor("wo", [P, NPAIR * DM], bf, kind="ExternalInput")
    out_d = nc.dram_tensor("out", [S, DM], bf, kind="ExternalOutput")

    from contextlib import ExitStack
    with tile.TileContext(nc) as tc, ExitStack() as ctx:
        const = ctx.enter_context(tc.tile_pool(name="const", bufs=1))
        work = ctx.enter_context(tc.tile_pool(name="work", bufs=3))
        pt_pool = ctx.enter_context(tc.tile_pool(name="pt", bufs=2))
        st_pool = ctx.enter_context(tc.tile_pool(name="st", bufs=2, space="PSUM"))
        pj_pool = ctx.enter_context(tc.tile_pool(name="pj", bufs=2, space="PSUM"))
        z_pool = ctx.enter_context(tc.tile_pool(name="zp", bufs=2, space="PSUM"))

        # ---- constants / inputs to SBUF ----
        xT_sb = const.tile([P, MC * S], bf)
        # column-group DMAs so the first q-group's matmuls start early
        for g in range(4):
            nc.sync.dma_start(
                out=xT_sb.rearrange("p (c s) -> p c s", s=S)[:, :, g * 512:(g + 1) * 512],
                in_=xT_d.rearrange("p (c s) -> p c s", s=S)[:, :, g * 512:(g + 1) * 512],
            )
        wq_sb = const.tile([P, NPAIR * MC * P], bf)
        nc.sync.dma_start(out=wq_sb, in_=wq_d[:])
        wk_sb = const.tile([P, NPAIR * MC * P], bf)
        nc.sync.dma_start(out=wk_sb, in_=wk_d[:])
        wv_sb = const.tile([P, MC * HPC * DH], bf)
        nc.sync.dma_start(out=wv_sb, in_=wv_d[:])
        wo_sb = const.tile([P, NPAIR * DM], bf)
        nc.sync.dma_start(out=wo_sb, in_=wo_d[:])

        ident = const.tile([P, P], bf)
        make_identity(nc, ident)
        # causal keep-mask in [k, q] layout: 1 where k <= q else 0
        cmask = const.tile([P, P], bf)
        nc.gpsimd.memset(cmask, 1.0)
        nc.gpsimd.affine_select(
            out=cmask, in_=cmask,
            compare_op=mybir.AluOpType.is_ge,
            fill=0.0, base=0,
            pattern=[[1, P]],       # iota = q - k ; keep when >= 0
            channel_multiplier=-1,
        )

        qt_sb = const.tile([P, NPAIR * S], bf)   # Q^T per pair [2*64, S]
        kt_sb = const.tile([P, NPAIR * S], bf)
        vsb = const.tile([P, NKT * HPC * VSLOT], bf)
        nc.vector.memset(vsb, 1.0)               # ones survive in col 64 of each slot
        Zst = const.tile([P, NKT * HPC * DH], bf)

        # ---- V projection first (feeds every PV) ----
        for kt in range(NKT):
            ps = pj_pool.tile([P, 512], f32, tag="pj")
            for mc in range(MC):
                nc.tensor.matmul(
                    ps[:, 0:HPC * DH],
                    lhsT=xT_sb[:, mc * S + kt * P: mc * S + (kt + 1) * P],
                    rhs=wv_sb[:, mc * HPC * DH:(mc + 1) * HPC * DH],
                    start=(mc == 0), stop=(mc == MC - 1),
                )
            dst = vsb[:, kt * HPC * VSLOT:(kt + 1) * HPC * VSLOT]
            dst = dst.rearrange("p (h w) -> p h w", w=VSLOT)[:, :, 0:DH]
            src = ps[:, 0:HPC * DH].rearrange("p (h d) -> p h d", d=DH)
            nc.vector.tensor_copy(dst, src)

        # ---- per pair: Q^T/K^T projection, then pair-interleaved attention ----
        for p in range(NPAIR):
            for g in range(4):
                for wsb, dst in ((wq_sb, qt_sb), (wk_sb, kt_sb)):
                    ps = pj_pool.tile([P, 512], f32, tag="pj")
                    for mc in range(MC):
                        nc.tensor.matmul(
                            ps[:, 0:512],
                            lhsT=wsb[:, (p * MC + mc) * P:(p * MC + mc + 1) * P],
                            rhs=xT_sb[:, mc * S + g * 512: mc * S + g * 512 + 512],
                            start=(mc == 0), stop=(mc == MC - 1),
                        )
                    nc.vector.tensor_copy(
                        dst[:, p * S + g * 512: p * S + g * 512 + 512], ps[:, 0:512]
                    )

            PTh = [pt_pool.tile([P, PT_W], bf, tag="pt", name=f"PT{p}_{i}") for i in range(2)]
            qo = p * S
            for ki in range(NKT):
                cols = S - ki * P
                # S^T for both heads of the pair, row-tiled (array rows 0-63 / 64-127)
                sts = []
                c0 = 0
                while c0 < cols:
                    w = min(1024, cols - c0)
                    pss = [st_pool.tile([P, 1024], f32, tag="st", name=f"st{ki}_{c0}_{i}") for i in range(2)]
                    for s0 in range(0, w, 512):
                        sw = min(512, w - s0)
                        for half in range(2):
                            po = DH * half
                            nc.tensor.matmul(
                                pss[half][:, s0:s0 + sw],
                                lhsT=kt_sb[po:po + DH, qo + ki * P: qo + (ki + 1) * P],
                                rhs=qt_sb[po:po + DH,
                                          qo + ki * P + c0 + s0: qo + ki * P + c0 + s0 + sw],
                                start=True, stop=True,
                            )
                    for half in range(2):
                        nc.scalar.activation(
                            out=PTh[half][:, PT_OFF[ki] + c0: PT_OFF[ki] + c0 + w],
                            in_=pss[half][:, 0:w], func=EXP,
                        )
                    c0 += w
                for half in range(2):
                    nc.vector.tensor_mul(
                        PTh[half][:, PT_OFF[ki]:PT_OFF[ki] + P],
                        PTh[half][:, PT_OFF[ki]:PT_OFF[ki] + P],
                        cmask,
                    )
                # PV for q-tile qt == ki, both heads
                qt = ki
                for half in range(2):
                    h = 2 * p + half
                    zt = z_pool.tile([P, P], f32, tag="z")
                    for k2 in range(qt + 1):
                        nc.tensor.matmul(
                            zt[:, 0:VSLOT],
                            lhsT=PTh[half][:, PT_OFF[k2] + (qt - k2) * P:
                                           PT_OFF[k2] + (qt - k2 + 1) * P],
                            rhs=vsb[:, (k2 * HPC + h) * VSLOT:(k2 * HPC + h + 1) * VSLOT],
                            start=(k2 == 0), stop=(k2 == qt),
                        )
                    r = work.tile([P, 1], f32, tag="r")
                    nc.vector.reciprocal(r, zt[:, DH:DH + 1])
                    nc.vector.tensor_scalar_mul(
                        Zst[:, (qt * HPC + h) * DH:(qt * HPC + h + 1) * DH],
                        zt[:, 0:DH], r[:, 0:1],
                    )

        # ---- phase 3: Z transpose + output projection ----
        for qt in range(NKT):
            zts = work.tile([P, NPAIR * P], bf, tag="zt")
            for c in range(NPAIR):
                trp = z_pool.tile([P, P], bf, tag="z")
                nc.tensor.transpose(
                    trp[:, 0:P],
                    Zst[:, qt * HPC * DH + c * P: qt * HPC * DH + (c + 1) * P],
                    ident,
                )
                nc.vector.tensor_copy(zts[:, c * P:(c + 1) * P], trp[:, 0:P])
            ops = [pj_pool.tile([P, 512], f32, tag="pj", name=f"op{qt}_{i}") for i in range(2)]
            for (op, n0, nw) in ((ops[0], 0, 512), (ops[1], 512, 256)):
                for c in range(NPAIR):
                    nc.tensor.matmul(
                        op[:, 0:nw],
                        lhsT=zts[:, c * P:(c + 1) * P],
                        rhs=wo_sb[:, c * DM + n0: c * DM + n0 + nw],
                        start=(c == 0), stop=(c == NPAIR - 1),
                    )
            osb = work.tile([P, DM], bf, tag="o")
            nc.vector.tensor_copy(osb[:, 0:512], ops[0][:, 0:512])
            nc.vector.tensor_copy(osb[:, 512:768], ops[1][:, 0:256])
            nc.sync.dma_start(out=out_d[qt * P:(qt + 1) * P, :], in_=osb)

    nc.compile()
    return nc


_CACHED_NC = None


def _get_nc():
    global _CACHED_NC
    if _CACHED_NC is None:
        nc = bacc.Bacc("TRN2", target_bir_lowering=False, debug=False,
                       num_devices=NCORES)
        _CACHED_NC = build(nc)
    return _CACHED_NC


def _prep_core_inputs(x, W_Q, W_K, W_V, W_O, b_Q, b_K, b_V):
    """Host-side shard prep for one (batch, head-group) core.

    x: [S, DM] f32; W_*: [6, DM, DH] (W_O: [6, DH, DM]); b_*: [6, DH].
    Returns dict of bf16 SBUF-image arrays.
    """
    scale = 1.0 / np.sqrt(np.float32(DH))

    xT_aug = np.zeros((MC * P, S), np.float32)
    xT_aug[:DM] = x.T
    xT_aug[DM] = 1.0                      # bias row

    def pack_pairs(W, b):                 # -> [P, NPAIR*MC*P]
        img = np.zeros((P, NPAIR * MC * P), np.float32)
        for p in range(NPAIR):
            aug = np.zeros((MC * P, P), np.float32)
            aug[:DM, 0:DH] = W[2 * p]
            aug[:DM, DH:2 * DH] = W[2 * p + 1]
            aug[DM, 0:DH] = b[2 * p]
            aug[DM, DH:2 * DH] = b[2 * p + 1]
            for mc in range(MC):
                img[:, (p * MC + mc) * P:(p * MC + mc + 1) * P] = aug[mc * P:(mc + 1) * P]
        return img

    wq_img = pack_pairs(W_Q * scale, b_Q * scale)
    wk_img = pack_pairs(W_K, b_K)

    wv_aug = np.zeros((MC * P, HPC * DH), np.float32)
    wv_aug[:DM] = np.concatenate([W_V[h] for h in range(HPC)], axis=1)
    wv_aug[DM] = b_V.reshape(-1)
    wv_img = np.zeros((P, MC * HPC * DH), np.float32)
    for mc in range(MC):
        wv_img[:, mc * HPC * DH:(mc + 1) * HPC * DH] = wv_aug[mc * P:(mc + 1) * P]

    wo_flat = np.concatenate([W_O[h] for h in range(HPC)], axis=0)  # [384, DM]
    wo_img = np.zeros((P, NPAIR * DM), np.float32)
    for c in range(NPAIR):
        wo_img[:, c * DM:(c + 1) * DM] = wo_flat[c * P:(c + 1) * P]

    return {
        "xT": xT_aug.reshape(MC, P, S).transpose(1, 0, 2).reshape(P, MC * S).astype(BF16),
        "wq": wq_img.astype(BF16),
        "wk": wk_img.astype(BF16),
        "wv": wv_img.astype(BF16),
        "wo": wo_img.astype(BF16),
    }


def kernel(normalized_resid_pre, W_Q, W_K, W_V, W_O, b_Q, b_K, b_V, b_O):
    x = np.asarray(normalized_resid_pre, np.float32)
    nc = _get_nc()

    in_maps = []
    for core in range(NCORES):
        b, t = divmod(core, 2)
        hs = slice(t * HPC, (t + 1) * HPC)
        in_maps.append(_prep_core_inputs(
            x[b], np.asarray(W_Q)[hs], np.asarray(W_K)[hs], np.asarray(W_V)[hs],
            np.asarray(W_O)[hs], np.asarray(b_Q)[hs], np.asarray(b_K)[hs],
            np.asarray(b_V)[hs],
        ))

    res = run_bass_kernel_spmd(nc, in_maps, core_ids=list(range(NCORES)))
    out = np.zeros((4, S, DM), np.float32)
    for b in range(4):
        out[b] = (res.results[2 * b]["out"].astype(np.float32)
                  + res.results[2 * b + 1]["out"].astype(np.float32))
        out[b] += np.asarray(b_O, np.float32)
    return out


# revision 10
# speedup vs baseline: 1.3527x; 1.1672x over previous
"""Causal multi-head attention on 8 TRN2 NeuronCores.

Problem: x[4,2048,768], 12 heads x 64 dim, causal softmax attention.
Sharding: TP2 x DP4 -- core c handles batch c//2 and heads (c%2)*6..+6.
Each core computes a partial output (sum over its 6 heads); the host sums
the two partials per batch and adds b_O.

All matmuls run in bf16 (fp32 PSUM accumulation). The 1/sqrt(d_head)
scale and the Q/K/V biases are folded into the weights host-side (biases
enter through an augmented all-ones contraction row of x^T).
"""

import numpy as np
import ml_dtypes

import concourse.bacc as bacc
import concourse.tile as tile
from concourse import mybir
from concourse.bass_utils import run_bass_kernel_spmd
from concourse.masks import make_identity

BF16 = ml_dtypes.bfloat16

P = 128          # partitions
S = 2048         # sequence length
DM = 768         # d_model
DH = 64          # d_head
HPC = 6          # heads per core
NPAIR = HPC // 2
MC = 7           # m-chunks of x^T (768 rows + 1 bias row, padded to 896)
NKT = S // P     # key tiles (16)
VSLOT = DH + 1   # per-(ktile, head) V slot width: 64 data + ones column
NCORES = 8

# PT row offsets: row ki holds S^T[k in ki-tile, q in [ki*128, S)]
PT_OFF = [0] * (NKT + 1)
for _ki in range(NKT):
    PT_OFF[_ki + 1] = PT_OFF[_ki] + (S - _ki * P)
PT_W = PT_OFF[NKT]  # 17408


def build(nc):
    bf = mybir.dt.bfloat16
    f32 = mybir.dt.float32
    EXP = mybir.ActivationFunctionType.Exp
    IDENT = mybir.ActivationFunctionType.Identity

    xT_d = nc.dram_tensor("xT", [P, MC * S], bf, kind="ExternalInput")
    wq_d = nc.dram_tensor("wq", [P, NPAIR * MC * P], bf, kind="ExternalInput")
    wk_d = nc.dram_tensor("wk", [P, NPAIR * MC * P], bf, kind="ExternalInput")
    wv_d = nc.dram_tensor("wv", [P, MC * HPC * DH], bf, kind="ExternalInput")
    wo_d = nc.dram_tensor("wo", [P, NPAIR * DM], bf, kind="ExternalInput")
    out_d = nc.dram_tensor("out", [S, DM], bf, kind="ExternalOutput")

    from contextlib import ExitStack
    with tile.TileContext(nc) as tc, ExitStack() as ctx:
        const = ctx.enter_context(tc.tile_pool(name="const", bufs=1))
        work = ctx.enter_context(tc.tile_pool(name="work", bufs=3))
        pt_pool = ctx.enter_context(tc.tile_pool(name="pt", bufs=2))
        st_pool = ctx.enter_context(tc.tile_pool(name="st", bufs=2, space="PSUM"))
        pj_pool = ctx.enter_context(tc.tile_pool(name="pj", bufs=2, space="PSUM"))
        z_pool = ctx.enter_context(tc.tile_pool(name="zp", bufs=2, space="PSUM"))

        # ---- constants / inputs to SBUF ----
        wq_sb = const.tile([P, NPAIR * MC * P], bf)
        nc.sync.dma_start(out=wq_sb, in_=wq_d[:])
        wk_sb = const.tile([P, NPAIR * MC * P], bf)
        nc.sync.dma_start(out=wk_sb, in_=wk_d[:])
        wv_sb = const.tile([P, MC * HPC * DH], bf)
        nc.sync.dma_start(out=wv_sb, in_=wv_d[:])
        wo_sb = const.tile([P, NPAIR * DM], bf)
        nc.sync.dma_start(out=wo_sb, in_=wo_d[:])
        xT_sb = const.tile([P, MC * S], bf)
        # column-group DMAs so the first q-group's matmuls start early
        for g in range(4):
            nc.sync.dma_start(
                out=xT_sb.rearrange("p (c s) -> p c s", s=S)[:, :, g * 512:(g + 1) * 512],
                in_=xT_d.rearrange("p (c s) -> p c s", s=S)[:, :, g * 512:(g + 1) * 512],
            )

        ident = const.tile([P, P], bf)
        make_identity(nc, ident)
        # causal keep-mask in [k, q] layout: 1 where k <= q else 0
        cmask = const.tile([P, P], bf)
        nc.gpsimd.memset(cmask, 1.0)
        nc.gpsimd.affine_select(
            out=cmask, in_=cmask,
            compare_op=mybir.AluOpType.is_ge,
            fill=0.0, base=0,
            pattern=[[1, P]],       # iota = q - k ; keep when >= 0
            channel_multiplier=-1,
        )

        qt_sb = const.tile([P, NPAIR * S], bf)   # Q^T per pair [2*64, S]
        kt_sb = const.tile([P, NPAIR * S], bf)
        vsb = const.tile([P, NKT * HPC * VSLOT], bf)
        nc.vector.memset(vsb, 1.0)               # ones survive in col 64 of each slot
        Zst = const.tile([P, NKT * HPC * DH], bf)

        # ---- emission helpers (PE filler work woven into attention loops) ----
        def emit_v(kt):
            ps = pj_pool.tile([P, 512], f32, tag="pj", name=f"vps{kt}")
            for mc in range(MC):
                nc.tensor.matmul(
                    ps[:, 0:HPC * DH],
                    lhsT=xT_sb[:, mc * S + kt * P: mc * S + (kt + 1) * P],
                    rhs=wv_sb[:, mc * HPC * DH:(mc + 1) * HPC * DH],
                    start=(mc == 0), stop=(mc == MC - 1),
                )
            dst = vsb[:, kt * HPC * VSLOT:(kt + 1) * HPC * VSLOT]
            dst = dst.rearrange("p (h w) -> p h w", w=VSLOT)[:, :, 0:DH]
            src = ps[:, 0:HPC * DH].rearrange("p (h d) -> p h d", d=DH)
            nc.vector.tensor_copy(dst, src)

        def emit_qkt(p, g, which):
            wsb, dst = ((wq_sb, qt_sb), (wk_sb, kt_sb))[which]
            ps = pj_pool.tile([P, 512], f32, tag="pj", name=f"qkps{p}_{g}_{which}")
            for mc in range(MC):
                nc.tensor.matmul(
                    ps[:, 0:512],
                    lhsT=wsb[:, (p * MC + mc) * P:(p * MC + mc + 1) * P],
                    rhs=xT_sb[:, mc * S + g * 512: mc * S + g * 512 + 512],
                    start=(mc == 0), stop=(mc == MC - 1),
                )
            nc.vector.tensor_copy(
                dst[:, p * S + g * 512: p * S + g * 512 + 512], ps[:, 0:512]
            )

        def emit_phase3(qt):
            zts = work.tile([P, NPAIR * P], bf, tag="zt", name=f"zts{qt}")
            for c in range(NPAIR):
                trp = z_pool.tile([P, P], bf, tag="z", name=f"trp{qt}_{c}")
                nc.tensor.transpose(
                    trp[:, 0:P],
                    Zst[:, qt * HPC * DH + c * P: qt * HPC * DH + (c + 1) * P],
                    ident,
                )
                nc.vector.tensor_copy(zts[:, c * P:(c + 1) * P], trp[:, 0:P])
            ops = [pj_pool.tile([P, 512], f32, tag="pj", name=f"op{qt}_{i}")
                   for i in range(2)]
            for (op, n0, nw) in ((ops[0], 0, 512), (ops[1], 512, 256)):
                for c in range(NPAIR):
                    nc.tensor.matmul(
                        op[:, 0:nw],
                        lhsT=zts[:, c * P:(c + 1) * P],
                        rhs=wo_sb[:, c * DM + n0: c * DM + n0 + nw],
                        start=(c == 0), stop=(c == NPAIR - 1),
                    )
            osb = work.tile([P, DM], bf, tag="o", name=f"osb{qt}")
            nc.vector.tensor_copy(osb[:, 0:512], ops[0][:, 0:512])
            nc.vector.tensor_copy(osb[:, 512:768], ops[1][:, 0:256])
            nc.sync.dma_start(out=out_d[qt * P:(qt + 1) * P, :], in_=osb)

        # ---- pair 0's Q/K projections up front, then pair-interleaved attention ----
        for g in range(4):
            for which in range(2):
                emit_qkt(0, g, which)
        emit_v(0)

        for p in range(NPAIR):
            PTh = [pt_pool.tile([P, PT_W], bf, tag="pt", name=f"PT{p}_{i}") for i in range(2)]
            qo = p * S
            for ki in range(NKT):
                cols = S - ki * P
                # S^T for both heads of the pair, row-tiled (array rows 0-63 / 64-127)
                c0 = 0
                while c0 < cols:
                    w = min(1024, cols - c0)
                    pss = [st_pool.tile([P, 1024], f32, tag="st",
                                        name=f"st{ki}_{c0}_{i}") for i in range(2)]
                    for s0 in range(0, w, 512):
                        sw = min(512, w - s0)
                        for half in range(2):
                            po = DH * half
                            nc.tensor.matmul(
                                pss[half][:, s0:s0 + sw],
                                lhsT=kt_sb[po:po + DH, qo + ki * P: qo + (ki + 1) * P],
                                rhs=qt_sb[po:po + DH,
                                          qo + ki * P + c0 + s0: qo + ki * P + c0 + s0 + sw],
                                start=True, stop=True,
                            )
                    for half in range(2):
                        nc.scalar.activation(
                            out=PTh[half][:, PT_OFF[ki] + c0: PT_OFF[ki] + c0 + w],
                            in_=pss[half][:, 0:w], func=EXP,
                        )
                    c0 += w
                # PE filler while ScalarE drains the exp backlog
                if p == 0 and ki + 1 < NKT:
                    emit_v(ki + 1)
                if p < NPAIR - 1 and ki >= 8:
                    emit_qkt(p + 1, (ki - 8) // 2, (ki - 8) % 2)
                if p == NPAIR - 1 and ki >= 1:
                    emit_phase3(ki - 1)
                for half in range(2):
                    nc.vector.tensor_mul(
                        PTh[half][:, PT_OFF[ki]:PT_OFF[ki] + P],
                        PTh[half][:, PT_OFF[ki]:PT_OFF[ki] + P],
                        cmask,
                    )
                # PV for q-tile qt == ki, both heads
                qt = ki
                for half in range(2):
                    h = 2 * p + half
                    zt = z_pool.tile([P, P], f32, tag="z", name=f"zt{p}_{ki}_{half}")
                    for k2 in range(qt + 1):
                        nc.tensor.matmul(
                            zt[:, 0:VSLOT],
                            lhsT=PTh[half][:, PT_OFF[k2] + (qt - k2) * P:
                                           PT_OFF[k2] + (qt - k2 + 1) * P],
                            rhs=vsb[:, (k2 * HPC + h) * VSLOT:(k2 * HPC + h + 1) * VSLOT],
                            start=(k2 == 0), stop=(k2 == qt),
                        )
                    r = work.tile([P, 1], f32, tag="r")
                    nc.vector.reciprocal(r, zt[:, DH:DH + 1])
                    nc.vector.tensor_scalar_mul(
                        Zst[:, (qt * HPC + h) * DH:(qt * HPC + h + 1) * DH],
                        zt[:, 0:DH], r[:, 0:1],
                    )
        emit_phase3(NKT - 1)

    nc.compile()
    return nc


_CACHED_NC = None


def _get_nc():
    global _CACHED_NC
    if _CACHED_NC is None:
        nc = bacc.Bacc("TRN2", target_bir_lowering=False, debug=False,
                       num_devices=NCORES)
        _CACHED_NC = build(nc)
    return _CACHED_NC


def _prep_core_inputs(x, W_Q, W_K, W_V, W_O, b_Q, b_K, b_V):
    """Host-side shard prep for one (batch, head-group) core.

    x: [S, DM] f32; W_*: [6, DM, DH] (W_O: [6, DH, DM]); b_*: [6, DH].
    Returns dict of bf16 SBUF-image arrays.
    """
    scale = 1.0 / np.sqrt(np.float32(DH))

    xT_aug = np.zeros((MC * P, S), np.float32)
    xT_aug[:DM] = x.T
    xT_aug[DM] = 1.0                      # bias row

    def pack_pairs(W, b):                 # -> [P, NPAIR*MC*P]
        img = np.zeros((P, NPAIR * MC * P), np.float32)
        for p in range(NPAIR):
            aug = np.zeros((MC * P, P), np.float32)
            aug[:DM, 0:DH] = W[2 * p]
            aug[:DM, DH:2 * DH] = W[2 * p + 1]
            aug[DM, 0:DH] = b[2 * p]
            aug[DM, DH:2 * DH] = b[2 * p + 1]
            for mc in range(MC):
                img[:, (p * MC + mc) * P:(p * MC + mc + 1) * P] = aug[mc * P:(mc + 1) * P]
        return img

    wq_img = pack_pairs(W_Q * scale, b_Q * scale)
    wk_img = pack_pairs(W_K, b_K)

    wv_aug = np.zeros((MC * P, HPC * DH), np.float32)
    wv_aug[:DM] = np.concatenate([W_V[h] for h in range(HPC)], axis=1)
    wv_aug[DM] = b_V.reshape(-1)
    wv_img = np.zeros((P, MC * HPC * DH), np.float32)
    for mc in range(MC):
        wv_img[:, mc * HPC * DH:(mc + 1) * HPC * DH] = wv_aug[mc * P:(mc + 1) * P]

    wo_flat = np.concatenate([W_O[h] for h in range(HPC)], axis=0)  # [384, DM]
    wo_img = np.zeros((P, NPAIR * DM), np.float32)
    for c in range(NPAIR):
        wo_img[:, c * DM:(c + 1) * DM] = wo_flat[c * P:(c + 1) * P]

    return {
        "xT": xT_aug.reshape(MC, P, S).transpose(1, 0, 2).reshape(P, MC * S).astype(BF16),
        "wq": wq_img.astype(BF16),
        "wk": wk_img.astype(BF16),
        "wv": wv_img.astype(BF16),
        "wo": wo_img.astype(BF16),
    }


def kernel(normalized_resid_pre, W_Q, W_K, W_V, W_O, b_Q, b_K, b_V, b_O):
    x = np.asarray(normalized_resid_pre, np.float32)
    nc = _get_nc()

    in_maps = []
    for core in range(NCORES):
        b, t = divmod(core, 2)
        hs = slice(t * HPC, (t + 1) * HPC)
        in_maps.append(_prep_core_inputs(
            x[b], np.asarray(W_Q)[hs], np.asarray(W_K)[hs], np.asarray(W_V)[hs],
            np.asarray(W_O)[hs], np.asarray(b_Q)[hs], np.asarray(b_K)[hs],
            np.asarray(b_V)[hs],
        ))

    res = run_bass_kernel_spmd(nc, in_maps, core_ids=list(range(NCORES)))
    out = np.zeros((4, S, DM), np.float32)
    for b in range(4):
        out[b] = (res.results[2 * b]["out"].astype(np.float32)
                  + res.results[2 * b + 1]["out"].astype(np.float32))
        out[b] += np.asarray(b_O, np.float32)
    return out


# revision 11
# speedup vs baseline: 1.3840x; 1.0231x over previous
"""Causal multi-head attention on 8 TRN2 NeuronCores.

Problem: x[4,2048,768], 12 heads x 64 dim, causal softmax attention.
Sharding: TP2 x DP4 -- core c handles batch c//2 and heads (c%2)*6..+6.
Each core computes a partial output (sum over its 6 heads); the host sums
the two partials per batch and adds b_O.

All matmuls run in bf16 (fp32 PSUM accumulation). The 1/sqrt(d_head)
scale and the Q/K/V biases are folded into the weights host-side (biases
enter through an augmented all-ones contraction row of x^T).
"""

import numpy as np
import ml_dtypes

import concourse.bacc as bacc
import concourse.tile as tile
from concourse import mybir
from concourse.bass_utils import run_bass_kernel_spmd
from concourse.masks import make_identity

BF16 = ml_dtypes.bfloat16

P = 128          # partitions
S = 2048         # sequence length
DM = 768         # d_model
DH = 64          # d_head
HPC = 6          # heads per core
NPAIR = HPC // 2
MC = 7           # m-chunks of x^T (768 rows + 1 bias row, padded to 896)
NKT = S // P     # key tiles (16)
VSLOT = DH + 1   # per-(ktile, head) V slot width: 64 data + ones column
NCORES = 8

# PT row offsets: row ki holds S^T[k in ki-tile, q in [ki*128, S)]
PT_OFF = [0] * (NKT + 1)
for _ki in range(NKT):
    PT_OFF[_ki + 1] = PT_OFF[_ki] + (S - _ki * P)
PT_W = PT_OFF[NKT]  # 17408


def build(nc):
    bf = mybir.dt.bfloat16
    f32 = mybir.dt.float32
    EXP = mybir.ActivationFunctionType.Exp
    IDENT = mybir.ActivationFunctionType.Identity

    xT_d = nc.dram_tensor("xT", [P, MC * S], bf, kind="ExternalInput")
    wq_d = nc.dram_tensor("wq", [P, NPAIR * MC * P], bf, kind="ExternalInput")
    wk_d = nc.dram_tensor("wk", [P, NPAIR * MC * P], bf, kind="ExternalInput")
    wv_d = nc.dram_tensor("wv", [P, MC * HPC * DH], bf, kind="ExternalInput")
    wo_d = nc.dram_tensor("wo", [P, NPAIR * DM], bf, kind="ExternalInput")
    out_d = nc.dram_tensor("out", [S, DM], bf, kind="ExternalOutput")

    from contextlib import ExitStack
    with tile.TileContext(nc) as tc, ExitStack() as ctx:
        const = ctx.enter_context(tc.tile_pool(name="const", bufs=1))
        work = ctx.enter_context(tc.tile_pool(name="work", bufs=3))
        pt_pool = ctx.enter_context(tc.tile_pool(name="pt", bufs=2))
        st_pool = ctx.enter_context(tc.tile_pool(name="st", bufs=2, space="PSUM"))
        pj_pool = ctx.enter_context(tc.tile_pool(name="pj", bufs=2, space="PSUM"))
        z_pool = ctx.enter_context(tc.tile_pool(name="zp", bufs=2, space="PSUM"))

        # ---- constants / inputs to SBUF ----
        # DMA order matters: Q/K weights, then x^T by column group (so the first
        # projection groups start ASAP), then V/O weights.
        wq_sb = const.tile([P, NPAIR * MC * P], bf)
        nc.sync.dma_start(out=wq_sb, in_=wq_d[:])
        wk_sb = const.tile([P, NPAIR * MC * P], bf)
        nc.sync.dma_start(out=wk_sb, in_=wk_d[:])
        xT_sb = const.tile([P, MC * S], bf)
        # chunk 6 of x^T is the bias row + zero padding: synthesize, don't DMA
        nc.vector.memset(xT_sb[:, (MC - 1) * S: MC * S], 0.0)
        nc.vector.memset(xT_sb[0:1, (MC - 1) * S: MC * S], 1.0)
        for g in range(4):
            nc.sync.dma_start(
                out=xT_sb.rearrange("p (c s) -> p c s", s=S)[:, 0:MC - 1,
                                                             g * 512:(g + 1) * 512],
                in_=xT_d.rearrange("p (c s) -> p c s", s=S)[:, 0:MC - 1,
                                                            g * 512:(g + 1) * 512],
            )
        wv_sb = const.tile([P, MC * HPC * DH], bf)
        nc.sync.dma_start(out=wv_sb, in_=wv_d[:])
        wo_sb = const.tile([P, NPAIR * DM], bf)
        nc.sync.dma_start(out=wo_sb, in_=wo_d[:])

        ident = const.tile([P, P], bf)
        make_identity(nc, ident)
        # causal keep-mask in [k, q] layout: 1 where k <= q else 0
        cmask = const.tile([P, P], bf)
        nc.gpsimd.memset(cmask, 1.0)
        nc.gpsimd.affine_select(
            out=cmask, in_=cmask,
            compare_op=mybir.AluOpType.is_ge,
            fill=0.0, base=0,
            pattern=[[1, P]],       # iota = q - k ; keep when >= 0
            channel_multiplier=-1,
        )

        qt_sb = const.tile([P, NPAIR * S], bf)   # Q^T per pair [2*64, S]
        kt_sb = const.tile([P, NPAIR * S], bf)
        vsb = const.tile([P, NKT * HPC * VSLOT], bf)
        nc.vector.memset(vsb, 1.0)               # ones survive in col 64 of each slot
        Zst = const.tile([P, NKT * HPC * DH], bf)

        # ---- emission helpers (PE filler work woven into attention loops) ----
        def emit_v(kt):
            ps = pj_pool.tile([P, 512], f32, tag="pj", name=f"vps{kt}")
            for mc in range(MC):
                nc.tensor.matmul(
                    ps[:, 0:HPC * DH],
                    lhsT=xT_sb[:, mc * S + kt * P: mc * S + (kt + 1) * P],
                    rhs=wv_sb[:, mc * HPC * DH:(mc + 1) * HPC * DH],
                    start=(mc == 0), stop=(mc == MC - 1),
                )
            dst = vsb[:, kt * HPC * VSLOT:(kt + 1) * HPC * VSLOT]
            dst = dst.rearrange("p (h w) -> p h w", w=VSLOT)[:, :, 0:DH]
            src = ps[:, 0:HPC * DH].rearrange("p (h d) -> p h d", d=DH)
            nc.vector.tensor_copy(dst, src)

        def emit_qkt(p, g, which):
            wsb, dst = ((wq_sb, qt_sb), (wk_sb, kt_sb))[which]
            ps = pj_pool.tile([P, 512], f32, tag="pj", name=f"qkps{p}_{g}_{which}")
            for mc in range(MC):
                nc.tensor.matmul(
                    ps[:, 0:512],
                    lhsT=wsb[:, (p * MC + mc) * P:(p * MC + mc + 1) * P],
                    rhs=xT_sb[:, mc * S + g * 512: mc * S + g * 512 + 512],
                    start=(mc == 0), stop=(mc == MC - 1),
                )
            nc.vector.tensor_copy(
                dst[:, p * S + g * 512: p * S + g * 512 + 512], ps[:, 0:512]
            )

        def emit_phase3(qt):
            zts = work.tile([P, NPAIR * P], bf, tag="zt", name=f"zts{qt}")
            for c in range(NPAIR):
                trp = z_pool.tile([P, P], bf, tag="z", name=f"trp{qt}_{c}")
                nc.tensor.transpose(
                    trp[:, 0:P],
                    Zst[:, qt * HPC * DH + c * P: qt * HPC * DH + (c + 1) * P],
                    ident,
                )
                nc.vector.tensor_copy(zts[:, c * P:(c + 1) * P], trp[:, 0:P])
            ops = [pj_pool.tile([P, 512], f32, tag="pj", name=f"op{qt}_{i}")
                   for i in range(2)]
            for (op, n0, nw) in ((ops[0], 0, 512), (ops[1], 512, 256)):
                for c in range(NPAIR):
                    nc.tensor.matmul(
                        op[:, 0:nw],
                        lhsT=zts[:, c * P:(c + 1) * P],
                        rhs=wo_sb[:, c * DM + n0: c * DM + n0 + nw],
                        start=(c == 0), stop=(c == NPAIR - 1),
                    )
            osb = work.tile([P, DM], bf, tag="o", name=f"osb{qt}")
            nc.vector.tensor_copy(osb[:, 0:512], ops[0][:, 0:512])
            nc.vector.tensor_copy(osb[:, 512:768], ops[1][:, 0:256])
            nc.sync.dma_start(out=out_d[qt * P:(qt + 1) * P, :], in_=osb)

        # ---- pair 0's Q/K projections up front, then pair-interleaved attention ----
        for g in range(4):
            for which in range(2):
                emit_qkt(0, g, which)
        emit_v(0)

        for p in range(NPAIR):
            PTh = [pt_pool.tile([P, PT_W], bf, tag="pt", name=f"PT{p}_{i}") for i in range(2)]
            qo = p * S
            for ki in range(NKT):
                cols = S - ki * P
                # S^T for both heads of the pair, row-tiled (array rows 0-63 / 64-127)
                c0 = 0
                while c0 < cols:
                    w = min(1024, cols - c0)
                    pss = [st_pool.tile([P, 1024], f32, tag="st",
                                        name=f"st{ki}_{c0}_{i}") for i in range(2)]
                    for s0 in range(0, w, 512):
                        sw = min(512, w - s0)
                        for half in range(2):
                            po = DH * half
                            nc.tensor.matmul(
                                pss[half][:, s0:s0 + sw],
                                lhsT=kt_sb[po:po + DH, qo + ki * P: qo + (ki + 1) * P],
                                rhs=qt_sb[po:po + DH,
                                          qo + ki * P + c0 + s0: qo + ki * P + c0 + s0 + sw],
                                start=True, stop=True,
                            )
                    for half in range(2):
                        nc.scalar.activation(
                            out=PTh[half][:, PT_OFF[ki] + c0: PT_OFF[ki] + c0 + w],
                            in_=pss[half][:, 0:w], func=EXP,
                        )
                    c0 += w
                # PE filler while ScalarE drains the exp backlog
                if p == 0 and ki + 1 < NKT:
                    emit_v(ki + 1)
                if p < NPAIR - 1 and ki >= 8:
                    emit_qkt(p + 1, (ki - 8) // 2, (ki - 8) % 2)
                if p == NPAIR - 1 and ki >= 1:
                    emit_phase3(ki - 1)
                for half in range(2):
                    nc.vector.tensor_mul(
                        PTh[half][:, PT_OFF[ki]:PT_OFF[ki] + P],
                        PTh[half][:, PT_OFF[ki]:PT_OFF[ki] + P],
                        cmask,
                    )
                # PV for q-tile qt == ki, both heads
                qt = ki
                for half in range(2):
                    h = 2 * p + half
                    zt = z_pool.tile([P, P], f32, tag="z", name=f"zt{p}_{ki}_{half}")
                    for k2 in range(qt + 1):
                        nc.tensor.matmul(
                            zt[:, 0:VSLOT],
                            lhsT=PTh[half][:, PT_OFF[k2] + (qt - k2) * P:
                                           PT_OFF[k2] + (qt - k2 + 1) * P],
                            rhs=vsb[:, (k2 * HPC + h) * VSLOT:(k2 * HPC + h + 1) * VSLOT],
                            start=(k2 == 0), stop=(k2 == qt),
                        )
                    r = work.tile([P, 1], f32, tag="r")
                    nc.vector.reciprocal(r, zt[:, DH:DH + 1])
                    nc.vector.tensor_scalar_mul(
                        Zst[:, (qt * HPC + h) * DH:(qt * HPC + h + 1) * DH],
                        zt[:, 0:DH], r[:, 0:1],
                    )
        emit_phase3(NKT - 1)

    nc.compile()
    return nc


_CACHED_NC = None


def _get_nc():
    global _CACHED_NC
    if _CACHED_NC is None:
        nc = bacc.Bacc("TRN2", target_bir_lowering=False, debug=False,
                       num_devices=NCORES)
        _CACHED_NC = build(nc)
    return _CACHED_NC


def _prep_core_inputs(x, W_Q, W_K, W_V, W_O, b_Q, b_K, b_V):
    """Host-side shard prep for one (batch, head-group) core.

    x: [S, DM] f32; W_*: [6, DM, DH] (W_O: [6, DH, DM]); b_*: [6, DH].
    Returns dict of bf16 SBUF-image arrays.
    """
    scale = 1.0 / np.sqrt(np.float32(DH))

    xT_aug = np.zeros((MC * P, S), np.float32)
    xT_aug[:DM] = x.T
    xT_aug[DM] = 1.0                      # bias row

    def pack_pairs(W, b):                 # -> [P, NPAIR*MC*P]
        img = np.zeros((P, NPAIR * MC * P), np.float32)
        for p in range(NPAIR):
            aug = np.zeros((MC * P, P), np.float32)
            aug[:DM, 0:DH] = W[2 * p]
            aug[:DM, DH:2 * DH] = W[2 * p + 1]
            aug[DM, 0:DH] = b[2 * p]
            aug[DM, DH:2 * DH] = b[2 * p + 1]
            for mc in range(MC):
                img[:, (p * MC + mc) * P:(p * MC + mc + 1) * P] = aug[mc * P:(mc + 1) * P]
        return img

    wq_img = pack_pairs(W_Q * scale, b_Q * scale)
    wk_img = pack_pairs(W_K, b_K)

    wv_aug = np.zeros((MC * P, HPC * DH), np.float32)
    wv_aug[:DM] = np.concatenate([W_V[h] for h in range(HPC)], axis=1)
    wv_aug[DM] = b_V.reshape(-1)
    wv_img = np.zeros((P, MC * HPC * DH), np.float32)
    for mc in range(MC):
        wv_img[:, mc * HPC * DH:(mc + 1) * HPC * DH] = wv_aug[mc * P:(mc + 1) * P]

    wo_flat = np.concatenate([W_O[h] for h in range(HPC)], axis=0)  # [384, DM]
    wo_img = np.zeros((P, NPAIR * DM), np.float32)
    for c in range(NPAIR):
        wo_img[:, c * DM:(c + 1) * DM] = wo_flat[c * P:(c + 1) * P]

    return {
        "xT": xT_aug.reshape(MC, P, S).transpose(1, 0, 2).reshape(P, MC * S).astype(BF16),
        "wq": wq_img.astype(BF16),
        "wk": wk_img.astype(BF16),
        "wv": wv_img.astype(BF16),
        "wo": wo_img.astype(BF16),
    }


def kernel(normalized_resid_pre, W_Q, W_K, W_V, W_O, b_Q, b_K, b_V, b_O):
    x = np.asarray(normalized_resid_pre, np.float32)
    nc = _get_nc()

    in_maps = []
    for core in range(NCORES):
        b, t = divmod(core, 2)
        hs = slice(t * HPC, (t + 1) * HPC)
        in_maps.append(_prep_core_inputs(
            x[b], np.asarray(W_Q)[hs], np.asarray(W_K)[hs], np.asarray(W_V)[hs],
            np.asarray(W_O)[hs], np.asarray(b_Q)[hs], np.asarray(b_K)[hs],
            np.asarray(b_V)[hs],
        ))

    res = run_bass_kernel_spmd(nc, in_maps, core_ids=list(range(NCORES)))
    out = np.zeros((4, S, DM), np.float32)
    for b in range(4):
        out[b] = (res.results[2 * b]["out"].astype(np.float32)
                  + res.results[2 * b + 1]["out"].astype(np.float32))
        out[b] += np.asarray(b_O, np.float32)
    return out


# revision 12
# speedup vs baseline: 1.4046x; 1.0149x over previous
"""Causal multi-head attention on 8 TRN2 NeuronCores.

Problem: x[4,2048,768], 12 heads x 64 dim, causal softmax attention.
Sharding: TP2 x DP4 -- core c handles batch c//2 and heads (c%2)*6..+6.
Each core computes a partial output (sum over its 6 heads); the host sums
the two partials per batch and adds b_O.

All matmuls run in bf16 (fp32 PSUM accumulation). The 1/sqrt(d_head)
scale and the Q/K/V biases are folded into the weights host-side (biases
enter through an augmented all-ones contraction row of x^T).
"""

import numpy as np
import ml_dtypes

import concourse.bacc as bacc
import concourse.tile as tile
from concourse import mybir
from concourse.bass_utils import run_bass_kernel_spmd
from concourse.masks import make_identity

BF16 = ml_dtypes.bfloat16

P = 128          # partitions
S = 2048         # sequence length
DM = 768         # d_model
DH = 64          # d_head
HPC = 6          # heads per core
NPAIR = HPC // 2
MC = 7           # m-chunks of x^T (768 rows + 1 bias row, padded to 896)
NKT = S // P     # key tiles (16)
VSLOT = DH + 1   # per-(ktile, head) V slot width: 64 data + ones column
NCORES = 8

# PT row offsets: row ki holds S^T[k in ki-tile, q in [ki*128, S)]
PT_OFF = [0] * (NKT + 1)
for _ki in range(NKT):
    PT_OFF[_ki + 1] = PT_OFF[_ki] + (S - _ki * P)
PT_W = PT_OFF[NKT]  # 17408


def build(nc):
    bf = mybir.dt.bfloat16
    f32 = mybir.dt.float32
    EXP = mybir.ActivationFunctionType.Exp
    IDENT = mybir.ActivationFunctionType.Identity

    xT_d = nc.dram_tensor("xT", [P, MC * S], bf, kind="ExternalInput")
    wq_d = nc.dram_tensor("wq", [P, NPAIR * MC * P], bf, kind="ExternalInput")
    wk_d = nc.dram_tensor("wk", [P, NPAIR * MC * P], bf, kind="ExternalInput")
    wv_d = nc.dram_tensor("wv", [P, MC * HPC * DH], bf, kind="ExternalInput")
    wo_d = nc.dram_tensor("wo", [P, NPAIR * DM], bf, kind="ExternalInput")
    out_d = nc.dram_tensor("out", [S, DM], bf, kind="ExternalOutput")

    from contextlib import ExitStack
    with tile.TileContext(nc) as tc, ExitStack() as ctx:
        const = ctx.enter_context(tc.tile_pool(name="const", bufs=1))
        work = ctx.enter_context(tc.tile_pool(name="work", bufs=3))
        pt_pool = ctx.enter_context(tc.tile_pool(name="pt", bufs=2))
        st_pool = ctx.enter_context(tc.tile_pool(name="st", bufs=2, space="PSUM"))
        pj_pool = ctx.enter_context(tc.tile_pool(name="pj", bufs=2, space="PSUM"))
        z_pool = ctx.enter_context(tc.tile_pool(name="zp", bufs=2, space="PSUM"))

        # ---- constants / inputs to SBUF ----
        # DMA order matters: Q/K weights, then x^T by column group (so the first
        # projection groups start ASAP), then V/O weights.
        wq_sb = const.tile([P, NPAIR * MC * P], bf)
        nc.sync.dma_start(out=wq_sb, in_=wq_d[:])
        wk_sb = const.tile([P, NPAIR * MC * P], bf)
        nc.sync.dma_start(out=wk_sb, in_=wk_d[:])
        xT_sb = const.tile([P, MC * S], bf)
        # chunk 6 of x^T is the bias row + zero padding: synthesize, don't DMA
        nc.vector.memset(xT_sb[:, (MC - 1) * S: MC * S], 0.0)
        nc.vector.memset(xT_sb[0:1, (MC - 1) * S: MC * S], 1.0)
        for g in range(4):
            nc.sync.dma_start(
                out=xT_sb.rearrange("p (c s) -> p c s", s=S)[:, 0:MC - 1,
                                                             g * 512:(g + 1) * 512],
                in_=xT_d.rearrange("p (c s) -> p c s", s=S)[:, 0:MC - 1,
                                                            g * 512:(g + 1) * 512],
            )
        wv_sb = const.tile([P, MC * HPC * DH], bf)
        nc.sync.dma_start(out=wv_sb, in_=wv_d[:])
        wo_sb = const.tile([P, NPAIR * DM], bf)
        nc.sync.dma_start(out=wo_sb, in_=wo_d[:])

        ident = const.tile([P, P], bf)
        make_identity(nc, ident)
        # causal keep-mask in [k, q] layout: 1 where k <= q else 0
        cmask = const.tile([P, P], bf)
        nc.gpsimd.memset(cmask, 1.0)
        nc.gpsimd.affine_select(
            out=cmask, in_=cmask,
            compare_op=mybir.AluOpType.is_ge,
            fill=0.0, base=0,
            pattern=[[1, P]],       # iota = q - k ; keep when >= 0
            channel_multiplier=-1,
        )

        qt_sb = const.tile([P, NPAIR * S], bf)   # Q^T per pair [2*64, S]
        kt_sb = const.tile([P, NPAIR * S], bf)
        vsb = const.tile([P, NKT * HPC * VSLOT], bf)
        nc.vector.memset(vsb, 1.0)               # ones survive in col 64 of each slot
        Zst = const.tile([P, NKT * HPC * DH], bf)

        # ---- emission helpers (PE filler work woven into attention loops) ----
        def emit_v(kt):
            ps = pj_pool.tile([P, 512], f32, tag="pj", name=f"vps{kt}")
            for mc in range(MC):
                nc.tensor.matmul(
                    ps[:, 0:HPC * DH],
                    lhsT=xT_sb[:, mc * S + kt * P: mc * S + (kt + 1) * P],
                    rhs=wv_sb[:, mc * HPC * DH:(mc + 1) * HPC * DH],
                    start=(mc == 0), stop=(mc == MC - 1),
                )
            dst = vsb[:, kt * HPC * VSLOT:(kt + 1) * HPC * VSLOT]
            dst = dst.rearrange("p (h w) -> p h w", w=VSLOT)[:, :, 0:DH]
            src = ps[:, 0:HPC * DH].rearrange("p (h d) -> p h d", d=DH)
            nc.vector.tensor_copy(dst, src)

        def emit_qkt(p, g, which):
            wsb, dst = ((wq_sb, qt_sb), (wk_sb, kt_sb))[which]
            ps = pj_pool.tile([P, 512], f32, tag="pj", name=f"qkps{p}_{g}_{which}")
            for mc in range(MC):
                nc.tensor.matmul(
                    ps[:, 0:512],
                    lhsT=wsb[:, (p * MC + mc) * P:(p * MC + mc + 1) * P],
                    rhs=xT_sb[:, mc * S + g * 512: mc * S + g * 512 + 512],
                    start=(mc == 0), stop=(mc == MC - 1),
                )
            nc.vector.tensor_copy(
                dst[:, p * S + g * 512: p * S + g * 512 + 512], ps[:, 0:512]
            )

        def emit_phase3(qt):
            zts = work.tile([P, NPAIR * P], bf, tag="zt", name=f"zts{qt}")
            for c in range(NPAIR):
                trp = z_pool.tile([P, P], bf, tag="z", name=f"trp{qt}_{c}")
                nc.tensor.transpose(
                    trp[:, 0:P],
                    Zst[:, qt * HPC * DH + c * P: qt * HPC * DH + (c + 1) * P],
                    ident,
                )
                nc.vector.tensor_copy(zts[:, c * P:(c + 1) * P], trp[:, 0:P])
            ops = [pj_pool.tile([P, 512], f32, tag="pj", name=f"op{qt}_{i}")
                   for i in range(2)]
            for (op, n0, nw) in ((ops[0], 0, 512), (ops[1], 512, 256)):
                for c in range(NPAIR):
                    nc.tensor.matmul(
                        op[:, 0:nw],
                        lhsT=zts[:, c * P:(c + 1) * P],
                        rhs=wo_sb[:, c * DM + n0: c * DM + n0 + nw],
                        start=(c == 0), stop=(c == NPAIR - 1),
                    )
            osb = work.tile([P, DM], bf, tag="o", name=f"osb{qt}")
            nc.vector.tensor_copy(osb[:, 0:512], ops[0][:, 0:512])
            nc.vector.tensor_copy(osb[:, 512:768], ops[1][:, 0:256])
            nc.sync.dma_start(out=out_d[qt * P:(qt + 1) * P, :], in_=osb)

        # ---- pair 0's Q/K projections up front, then pair-interleaved attention ----
        for g in range(4):
            for which in range(2):
                emit_qkt(0, g, which)
        emit_v(0)

        for p in range(NPAIR):
            PTh = [pt_pool.tile([P, PT_W], bf, tag="pt", name=f"PT{p}_{i}") for i in range(2)]
            qo = p * S
            for ki in range(NKT):
                cols = S - ki * P
                # S^T for both heads of the pair, row-tiled (array rows 0-63 / 64-127)
                c0 = 0
                while c0 < cols:
                    w = min(1024, cols - c0)
                    pss = [st_pool.tile([P, 1024], f32, tag="st",
                                        name=f"st{ki}_{c0}_{i}") for i in range(2)]
                    for half in range(2):
                        po = DH * half
                        for s0 in range(0, w, 512):
                            sw = min(512, w - s0)
                            nc.tensor.matmul(
                                pss[half][:, s0:s0 + sw],
                                lhsT=kt_sb[po:po + DH, qo + ki * P: qo + (ki + 1) * P],
                                rhs=qt_sb[po:po + DH,
                                          qo + ki * P + c0 + s0: qo + ki * P + c0 + s0 + sw],
                                start=True, stop=True,
                            )
                    for half in range(2):
                        nc.scalar.activation(
                            out=PTh[half][:, PT_OFF[ki] + c0: PT_OFF[ki] + c0 + w],
                            in_=pss[half][:, 0:w], func=EXP,
                        )
                    c0 += w
                # PE filler while ScalarE drains the exp backlog
                if p == 0 and ki + 1 < NKT:
                    emit_v(ki + 1)
                if p < NPAIR - 1 and ki >= 8:
                    emit_qkt(p + 1, (ki - 8) // 2, (ki - 8) % 2)
                if p == NPAIR - 1 and ki >= 1:
                    emit_phase3(ki - 1)
                for half in range(2):
                    nc.vector.tensor_mul(
                        PTh[half][:, PT_OFF[ki]:PT_OFF[ki] + P],
                        PTh[half][:, PT_OFF[ki]:PT_OFF[ki] + P],
                        cmask,
                    )
                # PV for q-tile qt == ki, both heads
                qt = ki
                for half in range(2):
                    h = 2 * p + half
                    zt = z_pool.tile([P, P], f32, tag="z", name=f"zt{p}_{ki}_{half}")
                    for k2 in range(qt + 1):
                        nc.tensor.matmul(
                            zt[:, 0:VSLOT],
                            lhsT=PTh[half][:, PT_OFF[k2] + (qt - k2) * P:
                                           PT_OFF[k2] + (qt - k2 + 1) * P],
                            rhs=vsb[:, (k2 * HPC + h) * VSLOT:(k2 * HPC + h + 1) * VSLOT],
                            start=(k2 == 0), stop=(k2 == qt),
                        )
                    r = work.tile([P, 1], f32, tag="r")
                    nc.vector.reciprocal(r, zt[:, DH:DH + 1])
                    nc.vector.tensor_scalar_mul(
                        Zst[:, (qt * HPC + h) * DH:(qt * HPC + h + 1) * DH],
                        zt[:, 0:DH], r[:, 0:1],
                    )
        emit_phase3(NKT - 1)

    nc.compile()
    return nc


_CACHED_NC = None


def _get_nc():
    global _CACHED_NC
    if _CACHED_NC is None:
        nc = bacc.Bacc("TRN2", target_bir_lowering=False, debug=False,
                       num_devices=NCORES)
        _CACHED_NC = build(nc)
    return _CACHED_NC


def _prep_core_inputs(x, W_Q, W_K, W_V, W_O, b_Q, b_K, b_V):
    """Host-side shard prep for one (batch, head-group) core.

    x: [S, DM] f32; W_*: [6, DM, DH] (W_O: [6, DH, DM]); b_*: [6, DH].
    Returns dict of bf16 SBUF-image arrays.
    """
    scale = 1.0 / np.sqrt(np.float32(DH))

    xT_aug = np.zeros((MC * P, S), np.float32)
    xT_aug[:DM] = x.T
    xT_aug[DM] = 1.0                      # bias row

    def pack_pairs(W, b):                 # -> [P, NPAIR*MC*P]
        img = np.zeros((P, NPAIR * MC * P), np.float32)
        for p in range(NPAIR):
            aug = np.zeros((MC * P, P), np.float32)
            aug[:DM, 0:DH] = W[2 * p]
            aug[:DM, DH:2 * DH] = W[2 * p + 1]
            aug[DM, 0:DH] = b[2 * p]
            aug[DM, DH:2 * DH] = b[2 * p + 1]
            for mc in range(MC):
                img[:, (p * MC + mc) * P:(p * MC + mc + 1) * P] = aug[mc * P:(mc + 1) * P]
        return img

    wq_img = pack_pairs(W_Q * scale, b_Q * scale)
    wk_img = pack_pairs(W_K, b_K)

    wv_aug = np.zeros((MC * P, HPC * DH), np.float32)
    wv_aug[:DM] = np.concatenate([W_V[h] for h in range(HPC)], axis=1)
    wv_aug[DM] = b_V.reshape(-1)
    wv_img = np.zeros((P, MC * HPC * DH), np.float32)
    for mc in range(MC):
        wv_img[:, mc * HPC * DH:(mc + 1) * HPC * DH] = wv_aug[mc * P:(mc + 1) * P]

    wo_flat = np.concatenate([W_O[h] for h in range(HPC)], axis=0)  # [384, DM]
    wo_img = np.zeros((P, NPAIR * DM), np.float32)
    for c in range(NPAIR):
        wo_img[:, c * DM:(c + 1) * DM] = wo_flat[c * P:(c + 1) * P]

    return {
        "xT": xT_aug.reshape(MC, P, S).transpose(1, 0, 2).reshape(P, MC * S).astype(BF16),
        "wq": wq_img.astype(BF16),
        "wk": wk_img.astype(BF16),
        "wv": wv_img.astype(BF16),
        "wo": wo_img.astype(BF16),
    }


def kernel(normalized_resid_pre, W_Q, W_K, W_V, W_O, b_Q, b_K, b_V, b_O):
    x = np.asarray(normalized_resid_pre, np.float32)
    nc = _get_nc()

    in_maps = []
    for core in range(NCORES):
        b, t = divmod(core, 2)
        hs = slice(t * HPC, (t + 1) * HPC)
        in_maps.append(_prep_core_inputs(
            x[b], np.asarray(W_Q)[hs], np.asarray(W_K)[hs], np.asarray(W_V)[hs],
            np.asarray(W_O)[hs], np.asarray(b_Q)[hs], np.asarray(b_K)[hs],
            np.asarray(b_V)[hs],
        ))

    res = run_bass_kernel_spmd(nc, in_maps, core_ids=list(range(NCORES)))
    out = np.zeros((4, S, DM), np.float32)
    for b in range(4):
        out[b] = (res.results[2 * b]["out"].astype(np.float32)
                  + res.results[2 * b + 1]["out"].astype(np.float32))
        out[b] += np.asarray(b_O, np.float32)
    return out


# revision 13
# speedup vs baseline: 1.4166x; 1.0086x over previous
"""Causal multi-head attention on 8 TRN2 NeuronCores.

Problem: x[4,2048,768], 12 heads x 64 dim, causal softmax attention.
Sharding: TP2 x DP4 -- core c handles batch c//2 and heads (c%2)*6..+6.
Each core computes a partial output (sum over its 6 heads); the host sums
the two partials per batch and adds b_O.

All matmuls run in bf16 (fp32 PSUM accumulation). The 1/sqrt(d_head)
scale and the Q/K/V biases are folded into the weights host-side (biases
enter through an augmented all-ones contraction row of x^T).
"""

import numpy as np
import ml_dtypes

import concourse.bacc as bacc
import concourse.tile as tile
from concourse import mybir
from concourse.bass_utils import run_bass_kernel_spmd
from concourse.masks import make_identity

BF16 = ml_dtypes.bfloat16

P = 128          # partitions
S = 2048         # sequence length
DM = 768         # d_model
DH = 64          # d_head
HPC = 6          # heads per core
NPAIR = HPC // 2
MC = 7           # m-chunks of x^T (768 rows + 1 bias row, padded to 896)
NKT = S // P     # key tiles (16)
VSLOT = DH + 1   # per-(ktile, head) V slot width: 64 data + ones column
NCORES = 8

# PT row offsets: row ki holds S^T[k in ki-tile, q in [ki*128, S)]
PT_OFF = [0] * (NKT + 1)
for _ki in range(NKT):
    PT_OFF[_ki + 1] = PT_OFF[_ki] + (S - _ki * P)
PT_W = PT_OFF[NKT]  # 17408


def build(nc, mc=6):
    MC = mc
    bf = mybir.dt.bfloat16
    f32 = mybir.dt.float32
    EXP = mybir.ActivationFunctionType.Exp
    IDENT = mybir.ActivationFunctionType.Identity

    MC_ = mc
    xT_d = nc.dram_tensor("xT", [P, mc * S], bf, kind="ExternalInput")
    wq_d = nc.dram_tensor("wq", [P, NPAIR * mc * P], bf, kind="ExternalInput")
    wk_d = nc.dram_tensor("wk", [P, NPAIR * mc * P], bf, kind="ExternalInput")
    wv_d = nc.dram_tensor("wv", [P, mc * HPC * DH], bf, kind="ExternalInput")
    wo_d = nc.dram_tensor("wo", [P, NPAIR * DM], bf, kind="ExternalInput")
    out_d = nc.dram_tensor("out", [S, DM], bf, kind="ExternalOutput")

    from contextlib import ExitStack
    with tile.TileContext(nc) as tc, ExitStack() as ctx:
        const = ctx.enter_context(tc.tile_pool(name="const", bufs=1))
        work = ctx.enter_context(tc.tile_pool(name="work", bufs=3))
        pt_pool = ctx.enter_context(tc.tile_pool(name="pt", bufs=2))
        st_pool = ctx.enter_context(tc.tile_pool(name="st", bufs=2, space="PSUM"))
        pj_pool = ctx.enter_context(tc.tile_pool(name="pj", bufs=2, space="PSUM"))
        z_pool = ctx.enter_context(tc.tile_pool(name="zp", bufs=2, space="PSUM"))

        # ---- constants / inputs to SBUF ----
        # DMA order matters: Q/K weights, then x^T by column group (so the first
        # projection groups start ASAP), then V/O weights.
        wq_sb = const.tile([P, NPAIR * MC * P], bf)
        nc.sync.dma_start(out=wq_sb, in_=wq_d[:])
        wk_sb = const.tile([P, NPAIR * MC * P], bf)
        nc.sync.dma_start(out=wk_sb, in_=wk_d[:])
        xT_sb = const.tile([P, MC * S], bf)
        ndma = MC if MC == 6 else MC - 1
        if MC > 6:
            # chunk 6 of x^T is the bias row + zero padding: synthesize, don't DMA
            nc.vector.memset(xT_sb[:, (MC - 1) * S: MC * S], 0.0)
            nc.vector.memset(xT_sb[0:1, (MC - 1) * S: MC * S], 1.0)
        for g in range(4):
            nc.sync.dma_start(
                out=xT_sb.rearrange("p (c s) -> p c s", s=S)[:, 0:ndma,
                                                             g * 512:(g + 1) * 512],
                in_=xT_d.rearrange("p (c s) -> p c s", s=S)[:, 0:ndma,
                                                            g * 512:(g + 1) * 512],
            )
        wv_sb = const.tile([P, MC * HPC * DH], bf)
        nc.sync.dma_start(out=wv_sb, in_=wv_d[:])
        wo_sb = const.tile([P, NPAIR * DM], bf)
        nc.sync.dma_start(out=wo_sb, in_=wo_d[:])

        ident = const.tile([P, P], bf)
        make_identity(nc, ident)
        # causal keep-mask in [k, q] layout: 1 where k <= q else 0
        cmask = const.tile([P, P], bf)
        nc.gpsimd.memset(cmask, 1.0)
        nc.gpsimd.affine_select(
            out=cmask, in_=cmask,
            compare_op=mybir.AluOpType.is_ge,
            fill=0.0, base=0,
            pattern=[[1, P]],       # iota = q - k ; keep when >= 0
            channel_multiplier=-1,
        )

        qt_sb = const.tile([P, NPAIR * S], bf)   # Q^T per pair [2*64, S]
        kt_sb = const.tile([P, NPAIR * S], bf)
        vsb = const.tile([P, NKT * HPC * VSLOT], bf)
        nc.vector.memset(vsb, 1.0)               # ones survive in col 64 of each slot
        Zst = const.tile([P, NKT * HPC * DH], bf)

        # ---- emission helpers (PE filler work woven into attention loops) ----
        def emit_v(kt):
            ps = pj_pool.tile([P, 512], f32, tag="pj", name=f"vps{kt}")
            for mc in range(MC):
                nc.tensor.matmul(
                    ps[:, 0:HPC * DH],
                    lhsT=xT_sb[:, mc * S + kt * P: mc * S + (kt + 1) * P],
                    rhs=wv_sb[:, mc * HPC * DH:(mc + 1) * HPC * DH],
                    start=(mc == 0), stop=(mc == MC - 1),
                )
            dst = vsb[:, kt * HPC * VSLOT:(kt + 1) * HPC * VSLOT]
            dst = dst.rearrange("p (h w) -> p h w", w=VSLOT)[:, :, 0:DH]
            src = ps[:, 0:HPC * DH].rearrange("p (h d) -> p h d", d=DH)
            nc.vector.tensor_copy(dst, src)

        def emit_qkt(p, g, which):
            wsb, dst = ((wq_sb, qt_sb), (wk_sb, kt_sb))[which]
            ps = pj_pool.tile([P, 512], f32, tag="pj", name=f"qkps{p}_{g}_{which}")
            for mc in range(MC):
                nc.tensor.matmul(
                    ps[:, 0:512],
                    lhsT=wsb[:, (p * MC + mc) * P:(p * MC + mc + 1) * P],
                    rhs=xT_sb[:, mc * S + g * 512: mc * S + g * 512 + 512],
                    start=(mc == 0), stop=(mc == MC - 1),
                )
            nc.vector.tensor_copy(
                dst[:, p * S + g * 512: p * S + g * 512 + 512], ps[:, 0:512]
            )

        def emit_phase3(qt):
            zts = work.tile([P, NPAIR * P], bf, tag="zt", name=f"zts{qt}")
            for c in range(NPAIR):
                trp = z_pool.tile([P, P], bf, tag="z", name=f"trp{qt}_{c}")
                nc.tensor.transpose(
                    trp[:, 0:P],
                    Zst[:, qt * HPC * DH + c * P: qt * HPC * DH + (c + 1) * P],
                    ident,
                )
                nc.vector.tensor_copy(zts[:, c * P:(c + 1) * P], trp[:, 0:P])
            ops = [pj_pool.tile([P, 512], f32, tag="pj", name=f"op{qt}_{i}")
                   for i in range(2)]
            for (op, n0, nw) in ((ops[0], 0, 512), (ops[1], 512, 256)):
                for c in range(NPAIR):
                    nc.tensor.matmul(
                        op[:, 0:nw],
                        lhsT=zts[:, c * P:(c + 1) * P],
                        rhs=wo_sb[:, c * DM + n0: c * DM + n0 + nw],
                        start=(c == 0), stop=(c == NPAIR - 1),
                    )
            osb = work.tile([P, DM], bf, tag="o", name=f"osb{qt}")
            nc.vector.tensor_copy(osb[:, 0:512], ops[0][:, 0:512])
            nc.vector.tensor_copy(osb[:, 512:768], ops[1][:, 0:256])
            nc.sync.dma_start(out=out_d[qt * P:(qt + 1) * P, :], in_=osb)

        # ---- pair 0's Q/K projections up front, then pair-interleaved attention ----
        for g in range(4):
            for which in range(2):
                emit_qkt(0, g, which)
        emit_v(0)

        for p in range(NPAIR):
            PTh = [pt_pool.tile([P, PT_W], bf, tag="pt", name=f"PT{p}_{i}") for i in range(2)]
            qo = p * S
            for ki in range(NKT):
                cols = S - ki * P
                # S^T for both heads of the pair, row-tiled (array rows 0-63 / 64-127)
                c0 = 0
                while c0 < cols:
                    w = min(1024, cols - c0)
                    pss = [st_pool.tile([P, 1024], f32, tag="st",
                                        name=f"st{ki}_{c0}_{i}") for i in range(2)]
                    for half in range(2):
                        po = DH * half
                        for s0 in range(0, w, 512):
                            sw = min(512, w - s0)
                            nc.tensor.matmul(
                                pss[half][:, s0:s0 + sw],
                                lhsT=kt_sb[po:po + DH, qo + ki * P: qo + (ki + 1) * P],
                                rhs=qt_sb[po:po + DH,
                                          qo + ki * P + c0 + s0: qo + ki * P + c0 + s0 + sw],
                                start=True, stop=True,
                            )
                    for half in range(2):
                        nc.scalar.activation(
                            out=PTh[half][:, PT_OFF[ki] + c0: PT_OFF[ki] + c0 + w],
                            in_=pss[half][:, 0:w], func=EXP,
                        )
                    c0 += w
                # PE filler while ScalarE drains the exp backlog
                if p == 0 and ki + 1 < NKT:
                    emit_v(ki + 1)
                if p < NPAIR - 1 and ki >= 8:
                    emit_qkt(p + 1, (ki - 8) // 2, (ki - 8) % 2)
                if p == NPAIR - 1 and ki >= 1:
                    emit_phase3(ki - 1)
                for half in range(2):
                    nc.vector.tensor_mul(
                        PTh[half][:, PT_OFF[ki]:PT_OFF[ki] + P],
                        PTh[half][:, PT_OFF[ki]:PT_OFF[ki] + P],
                        cmask,
                    )
                # PV for q-tile qt == ki, both heads
                qt = ki
                for half in range(2):
                    h = 2 * p + half
                    zt = z_pool.tile([P, P], f32, tag="z", name=f"zt{p}_{ki}_{half}")
                    for k2 in range(qt + 1):
                        nc.tensor.matmul(
                            zt[:, 0:VSLOT],
                            lhsT=PTh[half][:, PT_OFF[k2] + (qt - k2) * P:
                                           PT_OFF[k2] + (qt - k2 + 1) * P],
                            rhs=vsb[:, (k2 * HPC + h) * VSLOT:(k2 * HPC + h + 1) * VSLOT],
                            start=(k2 == 0), stop=(k2 == qt),
                        )
                    r = work.tile([P, 1], f32, tag="r")
                    nc.vector.reciprocal(r, zt[:, DH:DH + 1])
                    nc.vector.tensor_scalar_mul(
                        Zst[:, (qt * HPC + h) * DH:(qt * HPC + h + 1) * DH],
                        zt[:, 0:DH], r[:, 0:1],
                    )
        emit_phase3(NKT - 1)

    nc.compile()
    return nc


_CACHED_NC = {}


def _get_nc(mc=6):
    if mc not in _CACHED_NC:
        nc = bacc.Bacc("TRN2", target_bir_lowering=False, debug=False,
                       num_devices=NCORES)
        _CACHED_NC[mc] = build(nc, mc=mc)
    return _CACHED_NC[mc]


def _prep_core_inputs(x, W_Q, W_K, W_V, W_O, b_Q, b_K, b_V, mc=6):
    """Host-side shard prep for one (batch, head-group) core.

    x: [S, DM] f32; W_*: [6, DM, DH] (W_O: [6, DH, DM]); b_*: [6, DH].
    Returns dict of bf16 SBUF-image arrays.
    """
    scale = 1.0 / np.sqrt(np.float32(DH))
    MC = mc

    xT_aug = np.zeros((MC * P, S), np.float32)
    xT_aug[:DM] = x.T
    if MC > 6:
        xT_aug[DM] = 1.0                  # bias row

    def pack_pairs(W, b):                 # -> [P, NPAIR*MC*P]
        img = np.zeros((P, NPAIR * MC * P), np.float32)
        for p in range(NPAIR):
            aug = np.zeros((MC * P, P), np.float32)
            aug[:DM, 0:DH] = W[2 * p]
            aug[:DM, DH:2 * DH] = W[2 * p + 1]
            if MC > 6:
                aug[DM, 0:DH] = b[2 * p]
                aug[DM, DH:2 * DH] = b[2 * p + 1]
            for mc in range(MC):
                img[:, (p * MC + mc) * P:(p * MC + mc + 1) * P] = aug[mc * P:(mc + 1) * P]
        return img

    wq_img = pack_pairs(W_Q * scale, b_Q * scale)
    wk_img = pack_pairs(W_K, b_K)

    wv_aug = np.zeros((MC * P, HPC * DH), np.float32)
    wv_aug[:DM] = np.concatenate([W_V[h] for h in range(HPC)], axis=1)
    if MC > 6:
        wv_aug[DM] = b_V.reshape(-1)
    wv_img = np.zeros((P, MC * HPC * DH), np.float32)
    for mc in range(MC):
        wv_img[:, mc * HPC * DH:(mc + 1) * HPC * DH] = wv_aug[mc * P:(mc + 1) * P]

    wo_flat = np.concatenate([W_O[h] for h in range(HPC)], axis=0)  # [384, DM]
    wo_img = np.zeros((P, NPAIR * DM), np.float32)
    for c in range(NPAIR):
        wo_img[:, c * DM:(c + 1) * DM] = wo_flat[c * P:(c + 1) * P]

    return {
        "xT": xT_aug.reshape(MC, P, S).transpose(1, 0, 2).reshape(P, MC * S).astype(BF16),
        "wq": wq_img.astype(BF16),
        "wk": wk_img.astype(BF16),
        "wv": wv_img.astype(BF16),
        "wo": wo_img.astype(BF16),
    }


def kernel(normalized_resid_pre, W_Q, W_K, W_V, W_O, b_Q, b_K, b_V, b_O):
    x = np.asarray(normalized_resid_pre, np.float32)
    mc = 6 if not (np.any(b_Q) or np.any(b_K) or np.any(b_V)) else 7
    nc = _get_nc(mc)

    in_maps = []
    for core in range(NCORES):
        b, t = divmod(core, 2)
        hs = slice(t * HPC, (t + 1) * HPC)
        in_maps.append(_prep_core_inputs(
            x[b], np.asarray(W_Q)[hs], np.asarray(W_K)[hs], np.asarray(W_V)[hs],
            np.asarray(W_O)[hs], np.asarray(b_Q)[hs], np.asarray(b_K)[hs],
            np.asarray(b_V)[hs], mc=mc,
        ))

    res = run_bass_kernel_spmd(nc, in_maps, core_ids=list(range(NCORES)))
    out = np.zeros((4, S, DM), np.float32)
    for b in range(4):
        out[b] = (res.results[2 * b]["out"].astype(np.float32)
                  + res.results[2 * b + 1]["out"].astype(np.float32))
        out[b] += np.asarray(b_O, np.float32)
    return out


# revision 14
# speedup vs baseline: 1.6234x; 1.1460x over previous
"""Causal multi-head attention on 8 TRN2 NeuronCores.

Problem: x[4,2048,768], 12 heads x 64 dim, causal softmax attention.
Sharding: TP2 x DP4 -- core c handles batch c//2 and heads (c%2)*6..+6.
Each core computes a partial output (sum over its 6 heads); the host sums
the two partials per batch and adds b_O.

All matmuls run in bf16 (fp32 PSUM accumulation). The 1/sqrt(d_head)
scale and the Q/K/V biases are folded into the weights host-side (biases
enter through an augmented all-ones contraction row of x^T).
"""

import numpy as np
import ml_dtypes

import concourse.bacc as bacc
import concourse.tile as tile
from concourse import mybir
from concourse.bass_utils import run_bass_kernel_spmd
from concourse.masks import make_identity

BF16 = ml_dtypes.bfloat16

P = 128          # partitions
S = 2048         # sequence length
DM = 768         # d_model
DH = 64          # d_head
HPC = 6          # heads per core
NPAIR = HPC // 2
MC = 7           # m-chunks of x^T (768 rows + 1 bias row, padded to 896)
NKT = S // P     # key tiles (16)
VSLOT = DH + 1   # per-(ktile, head) V slot width: 64 data + ones column
NCORES = 8

# PT row offsets: row ki holds S^T[k in ki-tile, q in [ki*128, S)]
PT_OFF = [0] * (NKT + 1)
for _ki in range(NKT):
    PT_OFF[_ki + 1] = PT_OFF[_ki] + (S - _ki * P)
PT_W = PT_OFF[NKT]  # 17408


def build(nc, mc=6):
    MC = mc
    bf = mybir.dt.bfloat16
    f32 = mybir.dt.float32
    EXP = mybir.ActivationFunctionType.Exp
    IDENT = mybir.ActivationFunctionType.Identity

    MC_ = mc
    xT_d = nc.dram_tensor("xT", [P, mc * S], bf, kind="ExternalInput")
    wq_d = nc.dram_tensor("wq", [P, NPAIR * mc * P], bf, kind="ExternalInput")
    wk_d = nc.dram_tensor("wk", [P, NPAIR * mc * P], bf, kind="ExternalInput")
    wv_d = nc.dram_tensor("wv", [P, mc * HPC * DH], bf, kind="ExternalInput")
    wo_d = nc.dram_tensor("wo", [P, NPAIR * DM], bf, kind="ExternalInput")
    out_d = nc.dram_tensor("out", [S, DM], bf, kind="ExternalOutput")

    from contextlib import ExitStack
    with tile.TileContext(nc) as tc, ExitStack() as ctx:
        const = ctx.enter_context(tc.tile_pool(name="const", bufs=1))
        work = ctx.enter_context(tc.tile_pool(name="work", bufs=3))
        pt_pool = ctx.enter_context(tc.tile_pool(name="pt", bufs=2))
        st_pool = ctx.enter_context(tc.tile_pool(name="st", bufs=2, space="PSUM"))
        pj_pool = ctx.enter_context(tc.tile_pool(name="pj", bufs=2, space="PSUM"))
        z_pool = ctx.enter_context(tc.tile_pool(name="zp", bufs=2, space="PSUM"))

        # ---- constants / inputs to SBUF ----
        # DMA order matters: Q/K weights, then x^T by column group (so the first
        # projection groups start ASAP), then V/O weights.
        wq_sb = const.tile([P, NPAIR * MC * P], bf)
        wk_sb = const.tile([P, NPAIR * MC * P], bf)
        wv_sb = const.tile([P, MC * HPC * DH], bf)
        wo_sb = const.tile([P, NPAIR * DM], bf)
        xT_sb = const.tile([P, MC * S], bf)
        ndma = MC if MC == 6 else MC - 1
        if MC > 6:
            # chunk 6 of x^T is the bias row + zero padding: synthesize, don't DMA
            nc.vector.memset(xT_sb[:, (MC - 1) * S: MC * S], 0.0)
            nc.vector.memset(xT_sb[0:1, (MC - 1) * S: MC * S], 1.0)

        def xg_dma(g):
            nc.sync.dma_start(
                out=xT_sb.rearrange("p (c s) -> p c s", s=S)[:, 0:ndma,
                                                             g * 512:(g + 1) * 512],
                in_=xT_d.rearrange("p (c s) -> p c s", s=S)[:, 0:ndma,
                                                            g * 512:(g + 1) * 512],
            )
        nc.sync.dma_start(out=wq_sb, in_=wq_d[:])
        xg_dma(0)
        nc.sync.dma_start(out=wk_sb, in_=wk_d[:])
        xg_dma(1)
        nc.sync.dma_start(out=wv_sb, in_=wv_d[:])
        xg_dma(2)
        nc.sync.dma_start(out=wo_sb, in_=wo_d[:])
        xg_dma(3)

        ident = const.tile([P, P], bf)
        make_identity(nc, ident)
        # causal keep-mask in [k, q] layout: 1 where k <= q else 0
        cmask = const.tile([P, P], bf)
        nc.gpsimd.memset(cmask, 1.0)
        nc.gpsimd.affine_select(
            out=cmask, in_=cmask,
            compare_op=mybir.AluOpType.is_ge,
            fill=0.0, base=0,
            pattern=[[1, P]],       # iota = q - k ; keep when >= 0
            channel_multiplier=-1,
        )

        qt_sb = const.tile([P, NPAIR * S], bf)   # Q^T per pair [2*64, S]
        # K^T per pair, one zero-padded copy per head (keeps S^T matmuls at K=128
        # with full 128-col FWL weight loads; the zero rows annihilate the other head)
        kt_e = const.tile([P, NPAIR * S], bf)
        nc.vector.memset(kt_e[DH:P, :], 0.0)
        kt_o = const.tile([P, NPAIR * S], bf)
        nc.vector.memset(kt_o[0:DH, :], 0.0)
        vsb = const.tile([P, NKT * HPC * VSLOT], bf)
        nc.vector.memset(vsb, 1.0)               # ones survive in col 64 of each slot
        Zst = const.tile([P, NKT * HPC * DH], bf)

        # ---- emission helpers (PE filler work woven into attention loops) ----
        def emit_v(kt):
            ps = pj_pool.tile([P, 512], f32, tag="pj", name=f"vps{kt}")
            for mc in range(MC):
                nc.tensor.matmul(
                    ps[:, 0:HPC * DH],
                    lhsT=xT_sb[:, mc * S + kt * P: mc * S + (kt + 1) * P],
                    rhs=wv_sb[:, mc * HPC * DH:(mc + 1) * HPC * DH],
                    start=(mc == 0), stop=(mc == MC - 1),
                )
            dst = vsb[:, kt * HPC * VSLOT:(kt + 1) * HPC * VSLOT]
            dst = dst.rearrange("p (h w) -> p h w", w=VSLOT)[:, :, 0:DH]
            src = ps[:, 0:HPC * DH].rearrange("p (h d) -> p h d", d=DH)
            nc.vector.tensor_copy(dst, src)

        def emit_qkt(p, g, which):
            wsb = (wq_sb, wk_sb)[which]
            ps = pj_pool.tile([P, 512], f32, tag="pj", name=f"qkps{p}_{g}_{which}")
            for mc in range(MC):
                nc.tensor.matmul(
                    ps[:, 0:512],
                    lhsT=wsb[:, (p * MC + mc) * P:(p * MC + mc + 1) * P],
                    rhs=xT_sb[:, mc * S + g * 512: mc * S + g * 512 + 512],
                    start=(mc == 0), stop=(mc == MC - 1),
                )
            c = slice(p * S + g * 512, p * S + g * 512 + 512)
            if which == 0:
                nc.vector.tensor_copy(qt_sb[:, c], ps[:, 0:512])
            else:
                nc.vector.tensor_copy(kt_e[0:DH, c], ps[0:DH, 0:512])
                nc.vector.tensor_copy(kt_o[DH:P, c], ps[DH:P, 0:512])

        def emit_phase3(qt):
            zts = work.tile([P, NPAIR * P], bf, tag="zt", name=f"zts{qt}")
            for c in range(NPAIR):
                trp = z_pool.tile([P, P], bf, tag="z", name=f"trp{qt}_{c}")
                nc.tensor.transpose(
                    trp[:, 0:P],
                    Zst[:, qt * HPC * DH + c * P: qt * HPC * DH + (c + 1) * P],
                    ident,
                )
                nc.vector.tensor_copy(zts[:, c * P:(c + 1) * P], trp[:, 0:P])
            ops = [pj_pool.tile([P, 512], f32, tag="pj", name=f"op{qt}_{i}")
                   for i in range(2)]
            for (op, n0, nw) in ((ops[0], 0, 512), (ops[1], 512, 256)):
                for c in range(NPAIR):
                    nc.tensor.matmul(
                        op[:, 0:nw],
                        lhsT=zts[:, c * P:(c + 1) * P],
                        rhs=wo_sb[:, c * DM + n0: c * DM + n0 + nw],
                        start=(c == 0), stop=(c == NPAIR - 1),
                    )
            osb = work.tile([P, DM], bf, tag="o", name=f"osb{qt}")
            nc.vector.tensor_copy(osb[:, 0:512], ops[0][:, 0:512])
            nc.vector.tensor_copy(osb[:, 512:768], ops[1][:, 0:256])
            nc.sync.dma_start(out=out_d[qt * P:(qt + 1) * P, :], in_=osb)

        # ---- pair 0's Q/K projections up front, then pair-interleaved attention ----
        for g in range(4):
            for which in range(2):
                emit_qkt(0, g, which)
        emit_v(0)

        for p in range(NPAIR):
            PTh = [pt_pool.tile([P, PT_W], bf, tag="pt", name=f"PT{p}_{i}") for i in range(2)]
            qo = p * S
            for ki in range(NKT):
                cols = S - ki * P
                # S^T for both heads of the pair, row-tiled (array rows 0-63 / 64-127)
                c0 = 0
                while c0 < cols:
                    w = min(1024, cols - c0)
                    pss = [st_pool.tile([P, 1024], f32, tag="st",
                                        name=f"st{ki}_{c0}_{i}") for i in range(2)]
                    for half in range(2):
                        ktx = (kt_e, kt_o)[half]
                        for s0 in range(0, w, 512):
                            sw = min(512, w - s0)
                            nc.tensor.matmul(
                                pss[half][:, s0:s0 + sw],
                                lhsT=ktx[:, qo + ki * P: qo + (ki + 1) * P],
                                rhs=qt_sb[:, qo + ki * P + c0 + s0:
                                          qo + ki * P + c0 + s0 + sw],
                                start=True, stop=True,
                            )
                    for half in range(2):
                        nc.scalar.activation(
                            out=PTh[half][:, PT_OFF[ki] + c0: PT_OFF[ki] + c0 + w],
                            in_=pss[half][:, 0:w], func=EXP,
                        )
                    c0 += w
                # PE filler while ScalarE drains the exp backlog
                if p == 0 and ki + 1 < NKT:
                    emit_v(ki + 1)
                if p < NPAIR - 1 and ki >= 8:
                    emit_qkt(p + 1, (ki - 8) // 2, (ki - 8) % 2)
                if p == NPAIR - 1 and ki >= 1:
                    emit_phase3(ki - 1)
                for half in range(2):
                    nc.vector.tensor_mul(
                        PTh[half][:, PT_OFF[ki]:PT_OFF[ki] + P],
                        PTh[half][:, PT_OFF[ki]:PT_OFF[ki] + P],
                        cmask,
                    )
                # PV for q-tile qt == ki, both heads
                qt = ki
                for half in range(2):
                    h = 2 * p + half
                    zt = z_pool.tile([P, P], f32, tag="z", name=f"zt{p}_{ki}_{half}")
                    for k2 in range(qt + 1):
                        nc.tensor.matmul(
                            zt[:, 0:VSLOT],
                            lhsT=PTh[half][:, PT_OFF[k2] + (qt - k2) * P:
                                           PT_OFF[k2] + (qt - k2 + 1) * P],
                            rhs=vsb[:, (k2 * HPC + h) * VSLOT:(k2 * HPC + h + 1) * VSLOT],
                            start=(k2 == 0), stop=(k2 == qt),
                        )
                    r = work.tile([P, 1], f32, tag="r")
                    nc.vector.reciprocal(r, zt[:, DH:DH + 1])
                    nc.vector.tensor_scalar_mul(
                        Zst[:, (qt * HPC + h) * DH:(qt * HPC + h + 1) * DH],
                        zt[:, 0:DH], r[:, 0:1],
                    )
        emit_phase3(NKT - 1)

    nc.compile()
    return nc


_CACHED_NC = {}


def _get_nc(mc=6):
    if mc not in _CACHED_NC:
        nc = bacc.Bacc("TRN2", target_bir_lowering=False, debug=False,
                       num_devices=NCORES)
        _CACHED_NC[mc] = build(nc, mc=mc)
    return _CACHED_NC[mc]


def _prep_core_inputs(x, W_Q, W_K, W_V, W_O, b_Q, b_K, b_V, mc=6):
    """Host-side shard prep for one (batch, head-group) core.

    x: [S, DM] f32; W_*: [6, DM, DH] (W_O: [6, DH, DM]); b_*: [6, DH].
    Returns dict of bf16 SBUF-image arrays.
    """
    scale = 1.0 / np.sqrt(np.float32(DH))
    MC = mc

    xT_aug = np.zeros((MC * P, S), np.float32)
    xT_aug[:DM] = x.T
    if MC > 6:
        xT_aug[DM] = 1.0                  # bias row

    def pack_pairs(W, b):                 # -> [P, NPAIR*MC*P]
        img = np.zeros((P, NPAIR * MC * P), np.float32)
        for p in range(NPAIR):
            aug = np.zeros((MC * P, P), np.float32)
            aug[:DM, 0:DH] = W[2 * p]
            aug[:DM, DH:2 * DH] = W[2 * p + 1]
            if MC > 6:
                aug[DM, 0:DH] = b[2 * p]
                aug[DM, DH:2 * DH] = b[2 * p + 1]
            for mc in range(MC):
                img[:, (p * MC + mc) * P:(p * MC + mc + 1) * P] = aug[mc * P:(mc + 1) * P]
        return img

    wq_img = pack_pairs(W_Q * scale, b_Q * scale)
    wk_img = pack_pairs(W_K, b_K)

    wv_aug = np.zeros((MC * P, HPC * DH), np.float32)
    wv_aug[:DM] = np.concatenate([W_V[h] for h in range(HPC)], axis=1)
    if MC > 6:
        wv_aug[DM] = b_V.reshape(-1)
    wv_img = np.zeros((P, MC * HPC * DH), np.float32)
    for mc in range(MC):
        wv_img[:, mc * HPC * DH:(mc + 1) * HPC * DH] = wv_aug[mc * P:(mc + 1) * P]

    wo_flat = np.concatenate([W_O[h] for h in range(HPC)], axis=0)  # [384, DM]
    wo_img = np.zeros((P, NPAIR * DM), np.float32)
    for c in range(NPAIR):
        wo_img[:, c * DM:(c + 1) * DM] = wo_flat[c * P:(c + 1) * P]

    return {
        "xT": xT_aug.reshape(MC, P, S).transpose(1, 0, 2).reshape(P, MC * S).astype(BF16),
        "wq": wq_img.astype(BF16),
        "wk": wk_img.astype(BF16),
        "wv": wv_img.astype(BF16),
        "wo": wo_img.astype(BF16),
    }


def kernel(normalized_resid_pre, W_Q, W_K, W_V, W_O, b_Q, b_K, b_V, b_O):
    x = np.asarray(normalized_resid_pre, np.float32)
    mc = 6 if not (np.any(b_Q) or np.any(b_K) or np.any(b_V)) else 7
    nc = _get_nc(mc)

    in_maps = []
    for core in range(NCORES):
        b, t = divmod(core, 2)
        hs = slice(t * HPC, (t + 1) * HPC)
        in_maps.append(_prep_core_inputs(
            x[b], np.asarray(W_Q)[hs], np.asarray(W_K)[hs], np.asarray(W_V)[hs],
            np.asarray(W_O)[hs], np.asarray(b_Q)[hs], np.asarray(b_K)[hs],
            np.asarray(b_V)[hs], mc=mc,
        ))

    res = run_bass_kernel_spmd(nc, in_maps, core_ids=list(range(NCORES)))
    out = np.zeros((4, S, DM), np.float32)
    for b in range(4):
        out[b] = (res.results[2 * b]["out"].astype(np.float32)
                  + res.results[2 * b + 1]["out"].astype(np.float32))
        out[b] += np.asarray(b_O, np.float32)
    return out


# revision 16
# speedup vs baseline: 1.6271x; 1.0023x over previous
"""Causal multi-head attention on 8 TRN2 NeuronCores.

Problem: x[4,2048,768], 12 heads x 64 dim, causal softmax attention.
Sharding: TP2 x DP4 -- core c handles batch c//2 and heads (c%2)*6..+6.
Each core computes a partial output (sum over its 6 heads); the host sums
the two partials per batch and adds b_O.

All matmuls run in bf16 (fp32 PSUM accumulation). The 1/sqrt(d_head)
scale and the Q/K/V biases are folded into the weights host-side (biases
enter through an augmented all-ones contraction row of x^T).
"""

import numpy as np
import ml_dtypes

import concourse.bacc as bacc
import concourse.tile as tile
from concourse import mybir
from concourse.bass_utils import run_bass_kernel_spmd
from concourse.masks import make_identity

BF16 = ml_dtypes.bfloat16

P = 128          # partitions
S = 2048         # sequence length
DM = 768         # d_model
DH = 64          # d_head
HPC = 6          # heads per core
NPAIR = HPC // 2
MC = 7           # m-chunks of x^T (768 rows + 1 bias row, padded to 896)
NKT = S // P     # key tiles (16)
VSLOT = DH + 1   # per-(ktile, head) V slot width: 64 data + ones column
NCORES = 8

# PT row offsets: row ki holds S^T[k in ki-tile, q in [ki*128, S)]
PT_OFF = [0] * (NKT + 1)
for _ki in range(NKT):
    PT_OFF[_ki + 1] = PT_OFF[_ki] + (S - _ki * P)
PT_W = PT_OFF[NKT]  # 17408


def build(nc, mc=6):
    MC = mc
    bf = mybir.dt.bfloat16
    f32 = mybir.dt.float32
    EXP = mybir.ActivationFunctionType.Exp
    IDENT = mybir.ActivationFunctionType.Identity

    nd = mc if mc == 6 else mc - 1
    xT_d = nc.dram_tensor("xT", [P, nd * S], bf, kind="ExternalInput")
    wq_d = nc.dram_tensor("wq", [P, NPAIR * mc * P], bf, kind="ExternalInput")
    wk_d = nc.dram_tensor("wk", [P, NPAIR * mc * P], bf, kind="ExternalInput")
    wv_d = nc.dram_tensor("wv", [P, mc * HPC * DH], bf, kind="ExternalInput")
    wo_d = nc.dram_tensor("wo", [P, NPAIR * DM], bf, kind="ExternalInput")
    out_d = nc.dram_tensor("out", [S, DM], bf, kind="ExternalOutput")

    from contextlib import ExitStack
    with tile.TileContext(nc) as tc, ExitStack() as ctx:
        const = ctx.enter_context(tc.tile_pool(name="const", bufs=1))
        work = ctx.enter_context(tc.tile_pool(name="work", bufs=3))
        pt_pool = ctx.enter_context(tc.tile_pool(name="pt", bufs=2))
        st_pool = ctx.enter_context(tc.tile_pool(name="st", bufs=2, space="PSUM"))
        pj_pool = ctx.enter_context(tc.tile_pool(name="pj", bufs=2, space="PSUM"))
        z_pool = ctx.enter_context(tc.tile_pool(name="zp", bufs=2, space="PSUM"))

        # ---- constants / inputs to SBUF ----
        # DMA order matters: Q/K weights, then x^T by column group (so the first
        # projection groups start ASAP), then V/O weights.
        wq_sb = const.tile([P, NPAIR * MC * P], bf)
        wk_sb = const.tile([P, NPAIR * MC * P], bf)
        wv_sb = const.tile([P, MC * HPC * DH], bf)
        wo_sb = const.tile([P, NPAIR * DM], bf)
        xT_sb = const.tile([P, MC * 4 * 512], bf)
        ndma = MC if MC == 6 else MC - 1

        GW_FULL = MC * 512                   # sbuf columns per group block
        GW = ndma * 512                      # dma'd columns per group block

        def xg_dma(g):
            nc.sync.dma_start(out=xT_sb[:, g * GW_FULL: g * GW_FULL + GW],
                              in_=xT_d[:, g * GW:(g + 1) * GW])

        def xslice(mc, s0, sw):
            g, r = divmod(s0, 512)
            assert r + sw <= 512
            return xT_sb[:, g * GW_FULL + mc * 512 + r:
                         g * GW_FULL + mc * 512 + r + sw]
        nc.sync.dma_start(out=wq_sb, in_=wq_d[:])
        xg_dma(0)
        nc.sync.dma_start(out=wk_sb, in_=wk_d[:])
        xg_dma(1)
        nc.sync.dma_start(out=wv_sb, in_=wv_d[:])
        xg_dma(2)
        nc.sync.dma_start(out=wo_sb, in_=wo_d[:])
        xg_dma(3)
        if MC > 6:
            # bias chunk: slot `ndma` of each group block: zeros + ones row
            for g in range(4):
                nc.vector.memset(xT_sb[:, g * GW_FULL + ndma * 512:
                                       g * GW_FULL + MC * 512], 0.0)
                nc.vector.memset(xT_sb[0:1, g * GW_FULL + ndma * 512:
                                        g * GW_FULL + MC * 512], 1.0)

        ident = const.tile([P, P], bf)
        make_identity(nc, ident)
        # causal keep-mask in [k, q] layout: 1 where k <= q else 0
        cmask = const.tile([P, P], bf)
        nc.gpsimd.memset(cmask, 1.0)
        nc.gpsimd.affine_select(
            out=cmask, in_=cmask,
            compare_op=mybir.AluOpType.is_ge,
            fill=0.0, base=0,
            pattern=[[1, P]],       # iota = q - k ; keep when >= 0
            channel_multiplier=-1,
        )

        qt_sb = const.tile([P, NPAIR * S], bf)   # Q^T per pair [2*64, S]
        # K^T per pair, one zero-padded copy per head (keeps S^T matmuls at K=128
        # with full 128-col FWL weight loads; the zero rows annihilate the other head)
        kt_e = const.tile([P, NPAIR * S], bf)
        nc.vector.memset(kt_e[DH:P, :], 0.0)
        kt_o = const.tile([P, NPAIR * S], bf)
        nc.vector.memset(kt_o[0:DH, :], 0.0)
        vsb = const.tile([P, NKT * HPC * VSLOT], bf)
        nc.vector.memset(vsb, 1.0)               # ones survive in col 64 of each slot
        Zst = const.tile([P, NKT * HPC * DH], bf)

        # ---- emission helpers (PE filler work woven into attention loops) ----
        def emit_v(kt):
            ps = pj_pool.tile([P, 512], f32, tag="pj", name=f"vps{kt}")
            for mc in range(MC):
                nc.tensor.matmul(
                    ps[:, 0:HPC * DH],
                    lhsT=xslice(mc, kt * P, P),
                    rhs=wv_sb[:, mc * HPC * DH:(mc + 1) * HPC * DH],
                    start=(mc == 0), stop=(mc == MC - 1),
                )
            dst = vsb[:, kt * HPC * VSLOT:(kt + 1) * HPC * VSLOT]
            dst = dst.rearrange("p (h w) -> p h w", w=VSLOT)[:, :, 0:DH]
            src = ps[:, 0:HPC * DH].rearrange("p (h d) -> p h d", d=DH)
            nc.vector.tensor_copy(dst, src)

        def emit_qkt(p, g, which):
            wsb = (wq_sb, wk_sb)[which]
            ps = pj_pool.tile([P, 512], f32, tag="pj", name=f"qkps{p}_{g}_{which}")
            for mc in range(MC):
                nc.tensor.matmul(
                    ps[:, 0:512],
                    lhsT=wsb[:, (p * MC + mc) * P:(p * MC + mc + 1) * P],
                    rhs=xslice(mc, g * 512, 512),
                    start=(mc == 0), stop=(mc == MC - 1),
                )
            c = slice(p * S + g * 512, p * S + g * 512 + 512)
            if which == 0:
                nc.vector.tensor_copy(qt_sb[:, c], ps[:, 0:512])
            else:
                nc.vector.tensor_copy(kt_e[0:DH, c], ps[0:DH, 0:512])
                nc.vector.tensor_copy(kt_o[DH:P, c], ps[DH:P, 0:512])

        def emit_phase3(qt):
            zts = work.tile([P, NPAIR * P], bf, tag="zt", name=f"zts{qt}")
            for c in range(NPAIR):
                trp = z_pool.tile([P, P], bf, tag="z", name=f"trp{qt}_{c}")
                nc.tensor.transpose(
                    trp[:, 0:P],
                    Zst[:, qt * HPC * DH + c * P: qt * HPC * DH + (c + 1) * P],
                    ident,
                )
                nc.vector.tensor_copy(zts[:, c * P:(c + 1) * P], trp[:, 0:P])
            ops = [pj_pool.tile([P, 512], f32, tag="pj", name=f"op{qt}_{i}")
                   for i in range(2)]
            for (op, n0, nw) in ((ops[0], 0, 512), (ops[1], 512, 256)):
                for c in range(NPAIR):
                    nc.tensor.matmul(
                        op[:, 0:nw],
                        lhsT=zts[:, c * P:(c + 1) * P],
                        rhs=wo_sb[:, c * DM + n0: c * DM + n0 + nw],
                        start=(c == 0), stop=(c == NPAIR - 1),
                    )
            osb = work.tile([P, DM], bf, tag="o", name=f"osb{qt}")
            nc.vector.tensor_copy(osb[:, 0:512], ops[0][:, 0:512])
            nc.vector.tensor_copy(osb[:, 512:768], ops[1][:, 0:256])
            nc.sync.dma_start(out=out_d[qt * P:(qt + 1) * P, :], in_=osb)

        # ---- pair 0's Q/K projections up front, then pair-interleaved attention ----
        for which in range(2):
            emit_qkt(0, 0, which)
        for kt in range(4):
            emit_v(kt)
        for g in range(1, 4):
            for which in range(2):
                emit_qkt(0, g, which)

        for p in range(NPAIR):
            PTh = [pt_pool.tile([P, PT_W], bf, tag="pt", name=f"PT{p}_{i}") for i in range(2)]
            qo = p * S
            for ki in range(NKT):
                cols = S - ki * P
                # S^T for both heads of the pair, row-tiled (array rows 0-63 / 64-127)
                c0 = 0
                while c0 < cols:
                    w = min(1024, cols - c0)
                    pss = [st_pool.tile([P, 1024], f32, tag="st",
                                        name=f"st{ki}_{c0}_{i}") for i in range(2)]
                    for half in range(2):
                        ktx = (kt_e, kt_o)[half]
                        for s0 in range(0, w, 512):
                            sw = min(512, w - s0)
                            nc.tensor.matmul(
                                pss[half][:, s0:s0 + sw],
                                lhsT=ktx[:, qo + ki * P: qo + (ki + 1) * P],
                                rhs=qt_sb[:, qo + ki * P + c0 + s0:
                                          qo + ki * P + c0 + s0 + sw],
                                start=True, stop=True,
                            )
                    for half in range(2):
                        nc.scalar.activation(
                            out=PTh[half][:, PT_OFF[ki] + c0: PT_OFF[ki] + c0 + w],
                            in_=pss[half][:, 0:w], func=EXP,
                        )
                    c0 += w
                # PE filler while ScalarE drains the exp backlog
                if p == 0 and ki + 4 < NKT:
                    emit_v(ki + 4)
                if p < NPAIR - 1 and ki >= 8:
                    emit_qkt(p + 1, (ki - 8) // 2, (ki - 8) % 2)
                if p == NPAIR - 1 and ki >= 1:
                    emit_phase3(ki - 1)
                for half in range(2):
                    nc.vector.tensor_mul(
                        PTh[half][:, PT_OFF[ki]:PT_OFF[ki] + P],
                        PTh[half][:, PT_OFF[ki]:PT_OFF[ki] + P],
                        cmask,
                    )
                # PV for q-tile qt == ki, both heads
                qt = ki
                for half in range(2):
                    h = 2 * p + half
                    zt = z_pool.tile([P, P], f32, tag="z", name=f"zt{p}_{ki}_{half}")
                    for k2 in range(qt + 1):
                        nc.tensor.matmul(
                            zt[:, 0:VSLOT],
                            lhsT=PTh[half][:, PT_OFF[k2] + (qt - k2) * P:
                                           PT_OFF[k2] + (qt - k2 + 1) * P],
                            rhs=vsb[:, (k2 * HPC + h) * VSLOT:(k2 * HPC + h + 1) * VSLOT],
                            start=(k2 == 0), stop=(k2 == qt),
                        )
                    r = work.tile([P, 1], f32, tag="r")
                    nc.vector.reciprocal(r, zt[:, DH:DH + 1])
                    nc.vector.tensor_scalar_mul(
                        Zst[:, (qt * HPC + h) * DH:(qt * HPC + h + 1) * DH],
                        zt[:, 0:DH], r[:, 0:1],
                    )
        emit_phase3(NKT - 1)

    nc.compile()
    return nc


_CACHED_NC = {}


def _get_nc(mc=6):
    if mc not in _CACHED_NC:
        nc = bacc.Bacc("TRN2", target_bir_lowering=False, debug=False,
                       num_devices=NCORES)
        _CACHED_NC[mc] = build(nc, mc=mc)
    return _CACHED_NC[mc]


def _prep_core_inputs(x, W_Q, W_K, W_V, W_O, b_Q, b_K, b_V, mc=6):
    """Host-side shard prep for one (batch, head-group) core.

    x: [S, DM] f32; W_*: [6, DM, DH] (W_O: [6, DH, DM]); b_*: [6, DH].
    Returns dict of bf16 SBUF-image arrays.
    """
    scale = 1.0 / np.sqrt(np.float32(DH))
    MC = mc

    xT_aug = np.zeros((MC * P, S), np.float32)
    xT_aug[:DM] = x.T
    if MC > 6:
        xT_aug[DM] = 1.0                  # bias row

    def pack_pairs(W, b):                 # -> [P, NPAIR*MC*P]
        img = np.zeros((P, NPAIR * MC * P), np.float32)
        for p in range(NPAIR):
            aug = np.zeros((MC * P, P), np.float32)
            aug[:DM, 0:DH] = W[2 * p]
            aug[:DM, DH:2 * DH] = W[2 * p + 1]
            if MC > 6:
                aug[DM, 0:DH] = b[2 * p]
                aug[DM, DH:2 * DH] = b[2 * p + 1]
            for mc in range(MC):
                img[:, (p * MC + mc) * P:(p * MC + mc + 1) * P] = aug[mc * P:(mc + 1) * P]
        return img

    wq_img = pack_pairs(W_Q * scale, b_Q * scale)
    wk_img = pack_pairs(W_K, b_K)

    wv_aug = np.zeros((MC * P, HPC * DH), np.float32)
    wv_aug[:DM] = np.concatenate([W_V[h] for h in range(HPC)], axis=1)
    if MC > 6:
        wv_aug[DM] = b_V.reshape(-1)
    wv_img = np.zeros((P, MC * HPC * DH), np.float32)
    for mc in range(MC):
        wv_img[:, mc * HPC * DH:(mc + 1) * HPC * DH] = wv_aug[mc * P:(mc + 1) * P]

    wo_flat = np.concatenate([W_O[h] for h in range(HPC)], axis=0)  # [384, DM]
    wo_img = np.zeros((P, NPAIR * DM), np.float32)
    for c in range(NPAIR):
        wo_img[:, c * DM:(c + 1) * DM] = wo_flat[c * P:(c + 1) * P]

    ndma = 6
    xt_img = np.zeros((P, 4 * ndma * 512), np.float32)
    for g in range(4):
        for c in range(ndma):
            xt_img[:, (g * ndma + c) * 512:(g * ndma + c + 1) * 512] = \
                xT_aug[c * P:(c + 1) * P, g * 512:(g + 1) * 512]
    return {
        "xT": xt_img.astype(BF16),
        "wq": wq_img.astype(BF16),
        "wk": wk_img.astype(BF16),
        "wv": wv_img.astype(BF16),
        "wo": wo_img.astype(BF16),
    }


def kernel(normalized_resid_pre, W_Q, W_K, W_V, W_O, b_Q, b_K, b_V, b_O):
    x = np.asarray(normalized_resid_pre, np.float32)
    mc = 6 if not (np.any(b_Q) or np.any(b_K) or np.any(b_V)) else 7
    nc = _get_nc(mc)

    in_maps = []
    for core in range(NCORES):
        b, t = divmod(core, 2)
        hs = slice(t * HPC, (t + 1) * HPC)
        in_maps.append(_prep_core_inputs(
            x[b], np.asarray(W_Q)[hs], np.asarray(W_K)[hs], np.asarray(W_V)[hs],
            np.asarray(W_O)[hs], np.asarray(b_Q)[hs], np.asarray(b_K)[hs],
            np.asarray(b_V)[hs], mc=mc,
        ))

    res = run_bass_kernel_spmd(nc, in_maps, core_ids=list(range(NCORES)))
    out = np.zeros((4, S, DM), np.float32)
    for b in range(4):
        out[b] = (res.results[2 * b]["out"].astype(np.float32)
                  + res.results[2 * b + 1]["out"].astype(np.float32))
        out[b] += np.asarray(b_O, np.float32)
    return out


# revision 24
# speedup vs baseline: 1.7397x; 1.0692x over previous
"""Causal multi-head attention on 8 TRN2 NeuronCores.

Problem: x[4,2048,768], 12 heads x 64 dim, causal softmax attention.
Sharding: TP2 x DP4 -- core c handles batch c//2 and heads (c%2)*6..+6.
Each core computes a partial output (sum over its 6 heads); the host sums
the two partials per batch and adds b_O.

All matmuls run in bf16 (fp32 PSUM accumulation). The 1/sqrt(d_head)
scale and the Q/K/V biases are folded into the weights host-side (biases
enter through an augmented all-ones contraction row of x^T).
"""

import numpy as np
import ml_dtypes

import concourse.bacc as bacc
import concourse.tile as tile
from concourse import mybir
from concourse.bass_utils import run_bass_kernel_spmd
from concourse.masks import make_identity

BF16 = ml_dtypes.bfloat16

P = 128          # partitions
S = 2048         # sequence length
DM = 768         # d_model
DH = 64          # d_head
HPC = 6          # heads per core
NPAIR = HPC // 2
MC = 7           # m-chunks of x^T (768 rows + 1 bias row, padded to 896)
NKT = S // P     # key tiles (16)
VSLOT = DH + 1   # per-(ktile, head) V slot width: 64 data + ones column
NCORES = 8

# PT row offsets: row ki holds S^T[k in ki-tile, q in [ki*128, S)]
PT_OFF = [0] * (NKT + 1)
for _ki in range(NKT):
    PT_OFF[_ki + 1] = PT_OFF[_ki] + (S - _ki * P)
PT_W = PT_OFF[NKT]  # 17408


def build(nc, mc=6):
    MC = mc
    bf = mybir.dt.bfloat16
    f32 = mybir.dt.float32
    EXP = mybir.ActivationFunctionType.Exp
    IDENT = mybir.ActivationFunctionType.Identity

    nd = mc if mc == 6 else mc - 1
    xT_d = nc.dram_tensor("xT", [P, nd * S], bf, kind="ExternalInput")
    wq_d = nc.dram_tensor("wq", [P, NPAIR * mc * P], bf, kind="ExternalInput")
    wk_d = nc.dram_tensor("wk", [P, NPAIR * mc * P], bf, kind="ExternalInput")
    wv_d = nc.dram_tensor("wv", [P, mc * HPC * DH], bf, kind="ExternalInput")
    wo_d = nc.dram_tensor("wo", [P, NPAIR * DM], bf, kind="ExternalInput")
    out_d = nc.dram_tensor("out", [S, DM], bf, kind="ExternalOutput")

    from contextlib import ExitStack
    with tile.TileContext(nc) as tc, ExitStack() as ctx:
        const = ctx.enter_context(tc.tile_pool(name="const", bufs=1))
        work = ctx.enter_context(tc.tile_pool(name="work", bufs=3))
        pt_pool = ctx.enter_context(tc.tile_pool(name="pt", bufs=2))
        st_pool = ctx.enter_context(tc.tile_pool(name="st", bufs=2, space="PSUM"))
        pj_pool = ctx.enter_context(tc.tile_pool(name="pj", bufs=2, space="PSUM"))
        z_pool = ctx.enter_context(tc.tile_pool(name="zp", bufs=2, space="PSUM"))

        # ---- constants / inputs to SBUF ----
        # DMA order matters: Q/K weights, then x^T by column group (so the first
        # projection groups start ASAP), then V/O weights.
        wq_sb = const.tile([P, NPAIR * MC * P], bf)
        wk_sb = const.tile([P, NPAIR * MC * P], bf)
        wv_sb = const.tile([P, MC * HPC * DH], bf)
        wo_sb = const.tile([P, NPAIR * DM], bf)
        xT_sb = const.tile([P, MC * 4 * 512], bf)
        ndma = MC if MC == 6 else MC - 1

        GW_FULL = MC * 512                   # sbuf columns per group block
        GW = ndma * 512                      # dma'd columns per group block

        def xg_dma(g):
            nc.sync.dma_start(out=xT_sb[:, g * GW_FULL: g * GW_FULL + GW],
                              in_=xT_d[:, g * GW:(g + 1) * GW])

        def xslice(mc, s0, sw):
            g, r = divmod(s0, 512)
            assert r + sw <= 512
            return xT_sb[:, g * GW_FULL + mc * 512 + r:
                         g * GW_FULL + mc * 512 + r + sw]
        nc.sync.dma_start(out=wq_sb, in_=wq_d[:])
        xg_dma(0)
        nc.sync.dma_start(out=wk_sb, in_=wk_d[:])
        xg_dma(1)
        nc.sync.dma_start(out=wv_sb, in_=wv_d[:])
        xg_dma(2)
        nc.sync.dma_start(out=wo_sb, in_=wo_d[:])
        xg_dma(3)
        if MC > 6:
            # bias chunk: slot `ndma` of each group block: zeros + ones row
            for g in range(4):
                nc.vector.memset(xT_sb[:, g * GW_FULL + ndma * 512:
                                       g * GW_FULL + MC * 512], 0.0)
                nc.vector.memset(xT_sb[0:1, g * GW_FULL + ndma * 512:
                                        g * GW_FULL + MC * 512], 1.0)

        ident = const.tile([P, P], bf)
        make_identity(nc, ident)
        # causal keep-mask in [k, q] layout: 1 where k <= q else 0
        cmask = const.tile([P, P], bf)
        nc.gpsimd.memset(cmask, 1.0)
        nc.gpsimd.affine_select(
            out=cmask, in_=cmask,
            compare_op=mybir.AluOpType.is_ge,
            fill=0.0, base=0,
            pattern=[[1, P]],       # iota = q - k ; keep when >= 0
            channel_multiplier=-1,
        )

        qt_sb = const.tile([P, NPAIR * S], bf)   # Q^T per pair [2*64, S]
        # K^T per pair, one zero-padded copy per head (keeps S^T matmuls at K=128
        # with full 128-col FWL weight loads; the zero rows annihilate the other head)
        kt_e = const.tile([P, NPAIR * S], bf)
        nc.gpsimd.memset(kt_e[DH:P, :], 0.0)
        kt_o = const.tile([P, NPAIR * S], bf)
        nc.gpsimd.memset(kt_o[0:DH, :], 0.0)
        vsb = const.tile([P, NKT * HPC * VSLOT], bf)
        nc.vector.memset(vsb, 1.0)               # ones survive in col 64 of each slot
        Zst = const.tile([P, NKT * HPC * DH], bf)

        # ---- emission helpers (PE filler work woven into attention loops) ----
        def emit_v(kt):
            ps = pj_pool.tile([P, 512], f32, tag="pj", name=f"vps{kt}")
            for mc in range(MC):
                nc.tensor.matmul(
                    ps[:, 0:HPC * DH],
                    lhsT=xslice(mc, kt * P, P),
                    rhs=wv_sb[:, mc * HPC * DH:(mc + 1) * HPC * DH],
                    start=(mc == 0), stop=(mc == MC - 1),
                )
            dst = vsb[:, kt * HPC * VSLOT:(kt + 1) * HPC * VSLOT]
            dst = dst.rearrange("p (h w) -> p h w", w=VSLOT)[:, :, 0:DH]
            src = ps[:, 0:HPC * DH].rearrange("p (h d) -> p h d", d=DH)
            nc.vector.tensor_copy(dst, src)

        def emit_qkt(p, g, which):
            wsb = (wq_sb, wk_sb)[which]
            ps = pj_pool.tile([P, 512], f32, tag="pj", name=f"qkps{p}_{g}_{which}")
            for mc in range(MC):
                nc.tensor.matmul(
                    ps[:, 0:512],
                    lhsT=wsb[:, (p * MC + mc) * P:(p * MC + mc + 1) * P],
                    rhs=xslice(mc, g * 512, 512),
                    start=(mc == 0), stop=(mc == MC - 1),
                )
            c = slice(p * S + g * 512, p * S + g * 512 + 512)
            if which == 0:
                nc.vector.tensor_copy(qt_sb[:, c], ps[:, 0:512])
            else:
                nc.vector.tensor_copy(kt_e[0:DH, c], ps[0:DH, 0:512])
                nc.vector.tensor_copy(kt_o[DH:P, c], ps[DH:P, 0:512])

        def emit_phase3(qt):
            zts = work.tile([P, NPAIR * P], bf, tag="zt", name=f"zts{qt}")
            for c in range(NPAIR):
                trp = pj_pool.tile([P, P], bf, tag="pj", name=f"trp{qt}_{c}")
                nc.tensor.transpose(
                    trp[:, 0:P],
                    Zst[:, qt * HPC * DH + c * P: qt * HPC * DH + (c + 1) * P],
                    ident,
                )
                nc.vector.tensor_copy(zts[:, c * P:(c + 1) * P], trp[:, 0:P])
            ops = [pj_pool.tile([P, 512], f32, tag="pj", name=f"op{qt}_{i}")
                   for i in range(2)]
            for (op, n0, nw) in ((ops[0], 0, 512), (ops[1], 512, 256)):
                for c in range(NPAIR):
                    nc.tensor.matmul(
                        op[:, 0:nw],
                        lhsT=zts[:, c * P:(c + 1) * P],
                        rhs=wo_sb[:, c * DM + n0: c * DM + n0 + nw],
                        start=(c == 0), stop=(c == NPAIR - 1),
                    )
            osb = work.tile([P, DM], bf, tag="o", name=f"osb{qt}")
            nc.vector.tensor_copy(osb[:, 0:512], ops[0][:, 0:512])
            nc.vector.tensor_copy(osb[:, 512:768], ops[1][:, 0:256])
            nc.sync.dma_start(out=out_d[qt * P:(qt + 1) * P, :], in_=osb)

        # ---- pair 0's Q/K projections up front, then pair-interleaved attention ----
        for which in range(2):
            emit_qkt(0, 0, which)
        for kt in range(4):
            emit_v(kt)
        for g in range(1, 4):
            for which in range(2):
                emit_qkt(0, g, which)

        for p in range(NPAIR):
            PTh = [pt_pool.tile([P, PT_W], bf, tag="pt", name=f"PT{p}_{i}") for i in range(2)]
            qo = p * S
            for ki in range(NKT):
                cols = S - ki * P
                # S^T for both heads of the pair, row-tiled (array rows 0-63 / 64-127)
                c0 = 0
                while c0 < cols:
                    w = min(1024, cols - c0)
                    pss = [st_pool.tile([P, 1024], f32, tag="st",
                                        name=f"st{ki}_{c0}_{i}") for i in range(2)]
                    for half in range(2):
                        ktx = (kt_e, kt_o)[half]
                        for s0 in range(0, w, 512):
                            sw = min(512, w - s0)
                            nc.tensor.matmul(
                                pss[half][:, s0:s0 + sw],
                                lhsT=ktx[:, qo + ki * P: qo + (ki + 1) * P],
                                rhs=qt_sb[:, qo + ki * P + c0 + s0:
                                          qo + ki * P + c0 + s0 + sw],
                                start=True, stop=True,
                            )
                    for half in range(2):
                        nc.scalar.activation(
                            out=PTh[half][:, PT_OFF[ki] + c0: PT_OFF[ki] + c0 + w],
                            in_=pss[half][:, 0:w], func=EXP,
                        )
                    c0 += w
                # PE filler while ScalarE drains the exp backlog
                if p == 0 and ki + 4 < NKT:
                    emit_v(ki + 4)
                if p < NPAIR - 1 and ki >= 8:
                    emit_qkt(p + 1, (ki - 8) // 2, (ki - 8) % 2)
                if p == NPAIR - 1 and ki >= 1:
                    emit_phase3(ki - 1)
                for half in range(2):
                    nc.vector.tensor_mul(
                        PTh[half][:, PT_OFF[ki]:PT_OFF[ki] + P],
                        PTh[half][:, PT_OFF[ki]:PT_OFF[ki] + P],
                        cmask,
                    )
                # PV for q-tile qt == ki, both heads
                qt = ki
                for half in range(2):
                    h = 2 * p + half
                    zfull = z_pool.tile([P, 512], f32, tag="z", name=f"zt{p}_{ki}_{half}")
                    zt = zfull[:, P * (ki % 4): P * (ki % 4) + P]
                    for k2 in range(qt + 1):
                        nc.tensor.matmul(
                            zt[:, 0:VSLOT],
                            lhsT=PTh[half][:, PT_OFF[k2] + (qt - k2) * P:
                                           PT_OFF[k2] + (qt - k2 + 1) * P],
                            rhs=vsb[:, (k2 * HPC + h) * VSLOT:(k2 * HPC + h + 1) * VSLOT],
                            start=(k2 == 0), stop=(k2 == qt),
                        )
                    r = work.tile([P, 1], f32, tag="r")
                    nc.vector.reciprocal(r, zt[:, DH:DH + 1])
                    nc.vector.tensor_scalar_mul(
                        Zst[:, (qt * HPC + h) * DH:(qt * HPC + h + 1) * DH],
                        zt[:, 0:DH], r[:, 0:1],
                    )
        emit_phase3(NKT - 1)

    nc.compile()
    return nc


_CACHED_NC = {}


def _get_nc(mc=6):
    if mc not in _CACHED_NC:
        nc = bacc.Bacc("TRN2", target_bir_lowering=False, debug=False,
                       num_devices=NCORES)
        _CACHED_NC[mc] = build(nc, mc=mc)
    return _CACHED_NC[mc]


def _prep_core_inputs(x, W_Q, W_K, W_V, W_O, b_Q, b_K, b_V, mc=6):
    """Host-side shard prep for one (batch, head-group) core.

    x: [S, DM] f32; W_*: [6, DM, DH] (W_O: [6, DH, DM]); b_*: [6, DH].
    Returns dict of bf16 SBUF-image arrays.
    """
    scale = 1.0 / np.sqrt(np.float32(DH))
    MC = mc

    xT_aug = np.zeros((MC * P, S), np.float32)
    xT_aug[:DM] = x.T
    if MC > 6:
        xT_aug[DM] = 1.0                  # bias row

    def pack_pairs(W, b):                 # -> [P, NPAIR*MC*P]
        img = np.zeros((P, NPAIR * MC * P), np.float32)
        for p in range(NPAIR):
            aug = np.zeros((MC * P, P), np.float32)
            aug[:DM, 0:DH] = W[2 * p]
            aug[:DM, DH:2 * DH] = W[2 * p + 1]
            if MC > 6:
                aug[DM, 0:DH] = b[2 * p]
                aug[DM, DH:2 * DH] = b[2 * p + 1]
            for mc in range(MC):
                img[:, (p * MC + mc) * P:(p * MC + mc + 1) * P] = aug[mc * P:(mc + 1) * P]
        return img

    wq_img = pack_pairs(W_Q * scale, b_Q * scale)
    wk_img = pack_pairs(W_K, b_K)

    wv_aug = np.zeros((MC * P, HPC * DH), np.float32)
    wv_aug[:DM] = np.concatenate([W_V[h] for h in range(HPC)], axis=1)
    if MC > 6:
        wv_aug[DM] = b_V.reshape(-1)
    wv_img = np.zeros((P, MC * HPC * DH), np.float32)
    for mc in range(MC):
        wv_img[:, mc * HPC * DH:(mc + 1) * HPC * DH] = wv_aug[mc * P:(mc + 1) * P]

    wo_flat = np.concatenate([W_O[h] for h in range(HPC)], axis=0)  # [384, DM]
    wo_img = np.zeros((P, NPAIR * DM), np.float32)
    for c in range(NPAIR):
        wo_img[:, c * DM:(c + 1) * DM] = wo_flat[c * P:(c + 1) * P]

    ndma = 6
    xt_img = np.zeros((P, 4 * ndma * 512), np.float32)
    for g in range(4):
        for c in range(ndma):
            xt_img[:, (g * ndma + c) * 512:(g * ndma + c + 1) * 512] = \
                xT_aug[c * P:(c + 1) * P, g * 512:(g + 1) * 512]
    return {
        "xT": xt_img.astype(BF16),
        "wq": wq_img.astype(BF16),
        "wk": wk_img.astype(BF16),
        "wv": wv_img.astype(BF16),
        "wo": wo_img.astype(BF16),
    }


def kernel(normalized_resid_pre, W_Q, W_K, W_V, W_O, b_Q, b_K, b_V, b_O):
    x = np.asarray(normalized_resid_pre, np.float32)
    mc = 6 if not (np.any(b_Q) or np.any(b_K) or np.any(b_V)) else 7
    nc = _get_nc(mc)

    in_maps = []
    for core in range(NCORES):
        b, t = divmod(core, 2)
        hs = slice(t * HPC, (t + 1) * HPC)
        in_maps.append(_prep_core_inputs(
            x[b], np.asarray(W_Q)[hs], np.asarray(W_K)[hs], np.asarray(W_V)[hs],
            np.asarray(W_O)[hs], np.asarray(b_Q)[hs], np.asarray(b_K)[hs],
            np.asarray(b_V)[hs], mc=mc,
        ))

    res = run_bass_kernel_spmd(nc, in_maps, core_ids=list(range(NCORES)))
    out = np.zeros((4, S, DM), np.float32)
    for b in range(4):
        out[b] = (res.results[2 * b]["out"].astype(np.float32)
                  + res.results[2 * b + 1]["out"].astype(np.float32))
        out[b] += np.asarray(b_O, np.float32)
    return out


# revision 25
# speedup vs baseline: 1.7567x; 1.0098x over previous
"""Causal multi-head attention on 8 TRN2 NeuronCores.

Problem: x[4,2048,768], 12 heads x 64 dim, causal softmax attention.
Sharding: TP2 x DP4 -- core c handles batch c//2 and heads (c%2)*6..+6.
Each core computes a partial output (sum over its 6 heads); the host sums
the two partials per batch and adds b_O.

All matmuls run in bf16 (fp32 PSUM accumulation). The 1/sqrt(d_head)
scale and the Q/K/V biases are folded into the weights host-side (biases
enter through an augmented all-ones contraction row of x^T).
"""

import numpy as np
import ml_dtypes

import concourse.bacc as bacc
import concourse.tile as tile
from concourse import mybir
from concourse.bass_utils import run_bass_kernel_spmd
from concourse.masks import make_identity

BF16 = ml_dtypes.bfloat16

P = 128          # partitions
S = 2048         # sequence length
DM = 768         # d_model
DH = 64          # d_head
HPC = 6          # heads per core
NPAIR = HPC // 2
MC = 7           # m-chunks of x^T (768 rows + 1 bias row, padded to 896)
NKT = S // P     # key tiles (16)
VSLOT = DH + 1   # per-(ktile, head) V slot width: 64 data + ones column
NCORES = 8

# PT row offsets: row ki holds S^T[k in ki-tile, q in [ki*128, S)]
PT_OFF = [0] * (NKT + 1)
for _ki in range(NKT):
    PT_OFF[_ki + 1] = PT_OFF[_ki] + (S - _ki * P)
PT_W = PT_OFF[NKT]  # 17408


def build(nc, mc=6):
    MC = mc
    bf = mybir.dt.bfloat16
    f32 = mybir.dt.float32
    EXP = mybir.ActivationFunctionType.Exp
    IDENT = mybir.ActivationFunctionType.Identity

    nd = mc if mc == 6 else mc - 1
    xT_d = nc.dram_tensor("xT", [P, nd * S], bf, kind="ExternalInput")
    wq_d = nc.dram_tensor("wq", [P, NPAIR * mc * P], bf, kind="ExternalInput")
    wk_d = nc.dram_tensor("wk", [P, NPAIR * mc * P], bf, kind="ExternalInput")
    wv_d = nc.dram_tensor("wv", [P, mc * HPC * DH], bf, kind="ExternalInput")
    wo_d = nc.dram_tensor("wo", [P, NPAIR * DM], bf, kind="ExternalInput")
    out_d = nc.dram_tensor("out", [S, DM], bf, kind="ExternalOutput")

    from contextlib import ExitStack
    with tile.TileContext(nc) as tc, ExitStack() as ctx:
        const = ctx.enter_context(tc.tile_pool(name="const", bufs=1))
        work = ctx.enter_context(tc.tile_pool(name="work", bufs=5))
        pt_pool = ctx.enter_context(tc.tile_pool(name="pt", bufs=2))
        st_pool = ctx.enter_context(tc.tile_pool(name="st", bufs=2, space="PSUM"))
        pj_pool = ctx.enter_context(tc.tile_pool(name="pj", bufs=2, space="PSUM"))
        z_pool = ctx.enter_context(tc.tile_pool(name="zp", bufs=2, space="PSUM"))

        # ---- constants / inputs to SBUF ----
        # DMA order matters: Q/K weights, then x^T by column group (so the first
        # projection groups start ASAP), then V/O weights.
        wq_sb = const.tile([P, NPAIR * MC * P], bf)
        wk_sb = const.tile([P, NPAIR * MC * P], bf)
        wv_sb = const.tile([P, MC * HPC * DH], bf)
        wo_sb = const.tile([P, NPAIR * DM], bf)
        xT_sb = const.tile([P, MC * 4 * 512], bf)
        ndma = MC if MC == 6 else MC - 1

        GW_FULL = MC * 512                   # sbuf columns per group block
        GW = ndma * 512                      # dma'd columns per group block

        def xg_dma(g):
            nc.sync.dma_start(out=xT_sb[:, g * GW_FULL: g * GW_FULL + GW],
                              in_=xT_d[:, g * GW:(g + 1) * GW])

        def xslice(mc, s0, sw):
            g, r = divmod(s0, 512)
            assert r + sw <= 512
            return xT_sb[:, g * GW_FULL + mc * 512 + r:
                         g * GW_FULL + mc * 512 + r + sw]
        nc.sync.dma_start(out=wq_sb, in_=wq_d[:])
        xg_dma(0)
        nc.sync.dma_start(out=wk_sb, in_=wk_d[:])
        xg_dma(1)
        nc.sync.dma_start(out=wv_sb, in_=wv_d[:])
        xg_dma(2)
        nc.sync.dma_start(out=wo_sb, in_=wo_d[:])
        xg_dma(3)
        if MC > 6:
            # bias chunk: slot `ndma` of each group block: zeros + ones row
            for g in range(4):
                nc.vector.memset(xT_sb[:, g * GW_FULL + ndma * 512:
                                       g * GW_FULL + MC * 512], 0.0)
                nc.vector.memset(xT_sb[0:1, g * GW_FULL + ndma * 512:
                                        g * GW_FULL + MC * 512], 1.0)

        ident = const.tile([P, P], bf)
        make_identity(nc, ident)
        # causal keep-mask in [k, q] layout: 1 where k <= q else 0
        cmask = const.tile([P, P], bf)
        nc.gpsimd.memset(cmask, 1.0)
        nc.gpsimd.affine_select(
            out=cmask, in_=cmask,
            compare_op=mybir.AluOpType.is_ge,
            fill=0.0, base=0,
            pattern=[[1, P]],       # iota = q - k ; keep when >= 0
            channel_multiplier=-1,
        )

        qt_sb = const.tile([P, NPAIR * S], bf)   # Q^T per pair [2*64, S]
        # K^T per pair, one zero-padded copy per head (keeps S^T matmuls at K=128
        # with full 128-col FWL weight loads; the zero rows annihilate the other head)
        kt_e = const.tile([P, NPAIR * S], bf)
        nc.gpsimd.memset(kt_e[DH:P, :], 0.0)
        kt_o = const.tile([P, NPAIR * S], bf)
        nc.gpsimd.memset(kt_o[0:DH, :], 0.0)
        vsb = const.tile([P, NKT * HPC * VSLOT], bf)
        nc.vector.memset(vsb, 1.0)               # ones survive in col 64 of each slot
        Zst = const.tile([P, NKT * HPC * DH], bf)

        # ---- emission helpers (PE filler work woven into attention loops) ----
        def emit_v(kt):
            ps = pj_pool.tile([P, 512], f32, tag="pj", name=f"vps{kt}")
            for mc in range(MC):
                nc.tensor.matmul(
                    ps[:, 0:HPC * DH],
                    lhsT=xslice(mc, kt * P, P),
                    rhs=wv_sb[:, mc * HPC * DH:(mc + 1) * HPC * DH],
                    start=(mc == 0), stop=(mc == MC - 1),
                )
            dst = vsb[:, kt * HPC * VSLOT:(kt + 1) * HPC * VSLOT]
            dst = dst.rearrange("p (h w) -> p h w", w=VSLOT)[:, :, 0:DH]
            src = ps[:, 0:HPC * DH].rearrange("p (h d) -> p h d", d=DH)
            nc.vector.tensor_copy(dst, src)

        def emit_qkt(p, g, which):
            wsb = (wq_sb, wk_sb)[which]
            ps = pj_pool.tile([P, 512], f32, tag="pj", name=f"qkps{p}_{g}_{which}")
            for mc in range(MC):
                nc.tensor.matmul(
                    ps[:, 0:512],
                    lhsT=wsb[:, (p * MC + mc) * P:(p * MC + mc + 1) * P],
                    rhs=xslice(mc, g * 512, 512),
                    start=(mc == 0), stop=(mc == MC - 1),
                )
            c = slice(p * S + g * 512, p * S + g * 512 + 512)
            if which == 0:
                nc.vector.tensor_copy(qt_sb[:, c], ps[:, 0:512])
            else:
                nc.vector.tensor_copy(kt_e[0:DH, c], ps[0:DH, 0:512])
                nc.vector.tensor_copy(kt_o[DH:P, c], ps[DH:P, 0:512])

        def emit_phase3(qt):
            zts = work.tile([P, NPAIR * P], bf, tag="zt", name=f"zts{qt}")
            for c in range(NPAIR):
                trp = pj_pool.tile([P, P], bf, tag="pj", name=f"trp{qt}_{c}")
                nc.tensor.transpose(
                    trp[:, 0:P],
                    Zst[:, qt * HPC * DH + c * P: qt * HPC * DH + (c + 1) * P],
                    ident,
                )
                nc.vector.tensor_copy(zts[:, c * P:(c + 1) * P], trp[:, 0:P])
            ops = [pj_pool.tile([P, 512], f32, tag="pj", name=f"op{qt}_{i}")
                   for i in range(2)]
            for (op, n0, nw) in ((ops[0], 0, 512), (ops[1], 512, 256)):
                for c in range(NPAIR):
                    nc.tensor.matmul(
                        op[:, 0:nw],
                        lhsT=zts[:, c * P:(c + 1) * P],
                        rhs=wo_sb[:, c * DM + n0: c * DM + n0 + nw],
                        start=(c == 0), stop=(c == NPAIR - 1),
                    )
            osb = work.tile([P, DM], bf, tag="o", name=f"osb{qt}")
            nc.vector.tensor_copy(osb[:, 0:512], ops[0][:, 0:512])
            nc.vector.tensor_copy(osb[:, 512:768], ops[1][:, 0:256])
            nc.sync.dma_start(out=out_d[qt * P:(qt + 1) * P, :], in_=osb)

        # ---- pair 0's Q/K projections up front, then pair-interleaved attention ----
        for which in range(2):
            emit_qkt(0, 0, which)
        for kt in range(4):
            emit_v(kt)
        for g in range(1, 4):
            for which in range(2):
                emit_qkt(0, g, which)

        for p in range(NPAIR):
            PTh = [pt_pool.tile([P, PT_W], bf, tag="pt", name=f"PT{p}_{i}") for i in range(2)]
            qo = p * S
            for ki in range(NKT):
                cols = S - ki * P
                # S^T for both heads of the pair, row-tiled (array rows 0-63 / 64-127)
                c0 = 0
                while c0 < cols:
                    w = min(1024, cols - c0)
                    pss = [st_pool.tile([P, 1024], f32, tag="st",
                                        name=f"st{ki}_{c0}_{i}") for i in range(2)]
                    for half in range(2):
                        ktx = (kt_e, kt_o)[half]
                        for s0 in range(0, w, 512):
                            sw = min(512, w - s0)
                            nc.tensor.matmul(
                                pss[half][:, s0:s0 + sw],
                                lhsT=ktx[:, qo + ki * P: qo + (ki + 1) * P],
                                rhs=qt_sb[:, qo + ki * P + c0 + s0:
                                          qo + ki * P + c0 + s0 + sw],
                                start=True, stop=True,
                            )
                    for half in range(2):
                        nc.scalar.activation(
                            out=PTh[half][:, PT_OFF[ki] + c0: PT_OFF[ki] + c0 + w],
                            in_=pss[half][:, 0:w], func=EXP,
                        )
                    c0 += w
                # PE filler while ScalarE drains the exp backlog
                if p == 0 and ki + 4 < NKT:
                    emit_v(ki + 4)
                if p < NPAIR - 1 and ki >= 8:
                    emit_qkt(p + 1, (ki - 8) // 2, (ki - 8) % 2)
                if p == NPAIR - 1 and ki >= 1:
                    emit_phase3(ki - 1)
                for half in range(2):
                    nc.vector.tensor_mul(
                        PTh[half][:, PT_OFF[ki]:PT_OFF[ki] + P],
                        PTh[half][:, PT_OFF[ki]:PT_OFF[ki] + P],
                        cmask,
                    )
                # PV for q-tile qt == ki, both heads
                qt = ki
                for half in range(2):
                    h = 2 * p + half
                    zfull = z_pool.tile([P, 512], f32, tag="z", name=f"zt{p}_{ki}_{half}")
                    zt = zfull[:, P * (ki % 4): P * (ki % 4) + P]
                    for k2 in range(qt + 1):
                        nc.tensor.matmul(
                            zt[:, 0:VSLOT],
                            lhsT=PTh[half][:, PT_OFF[k2] + (qt - k2) * P:
                                           PT_OFF[k2] + (qt - k2 + 1) * P],
                            rhs=vsb[:, (k2 * HPC + h) * VSLOT:(k2 * HPC + h + 1) * VSLOT],
                            start=(k2 == 0), stop=(k2 == qt),
                        )
                    r = work.tile([P, 1], f32, tag="r")
                    nc.vector.reciprocal(r, zt[:, DH:DH + 1])
                    nc.vector.tensor_scalar_mul(
                        Zst[:, (qt * HPC + h) * DH:(qt * HPC + h + 1) * DH],
                        zt[:, 0:DH], r[:, 0:1],
                    )
        emit_phase3(NKT - 1)

    nc.compile()
    return nc


_CACHED_NC = {}


def _get_nc(mc=6):
    if mc not in _CACHED_NC:
        nc = bacc.Bacc("TRN2", target_bir_lowering=False, debug=False,
                       num_devices=NCORES)
        _CACHED_NC[mc] = build(nc, mc=mc)
    return _CACHED_NC[mc]


def _prep_core_inputs(x, W_Q, W_K, W_V, W_O, b_Q, b_K, b_V, mc=6):
    """Host-side shard prep for one (batch, head-group) core.

    x: [S, DM] f32; W_*: [6, DM, DH] (W_O: [6, DH, DM]); b_*: [6, DH].
    Returns dict of bf16 SBUF-image arrays.
    """
    scale = 1.0 / np.sqrt(np.float32(DH))
    MC = mc

    xT_aug = np.zeros((MC * P, S), np.float32)
    xT_aug[:DM] = x.T
    if MC > 6:
        xT_aug[DM] = 1.0                  # bias row

    def pack_pairs(W, b):                 # -> [P, NPAIR*MC*P]
        img = np.zeros((P, NPAIR * MC * P), np.float32)
        for p in range(NPAIR):
            aug = np.zeros((MC * P, P), np.float32)
            aug[:DM, 0:DH] = W[2 * p]
            aug[:DM, DH:2 * DH] = W[2 * p + 1]
            if MC > 6:
                aug[DM, 0:DH] = b[2 * p]
                aug[DM, DH:2 * DH] = b[2 * p + 1]
            for mc in range(MC):
                img[:, (p * MC + mc) * P:(p * MC + mc + 1) * P] = aug[mc * P:(mc + 1) * P]
        return img

    wq_img = pack_pairs(W_Q * scale, b_Q * scale)
    wk_img = pack_pairs(W_K, b_K)

    wv_aug = np.zeros((MC * P, HPC * DH), np.float32)
    wv_aug[:DM] = np.concatenate([W_V[h] for h in range(HPC)], axis=1)
    if MC > 6:
        wv_aug[DM] = b_V.reshape(-1)
    wv_img = np.zeros((P, MC * HPC * DH), np.float32)
    for mc in range(MC):
        wv_img[:, mc * HPC * DH:(mc + 1) * HPC * DH] = wv_aug[mc * P:(mc + 1) * P]

    wo_flat = np.concatenate([W_O[h] for h in range(HPC)], axis=0)  # [384, DM]
    wo_img = np.zeros((P, NPAIR * DM), np.float32)
    for c in range(NPAIR):
        wo_img[:, c * DM:(c + 1) * DM] = wo_flat[c * P:(c + 1) * P]

    ndma = 6
    xt_img = np.zeros((P, 4 * ndma * 512), np.float32)
    for g in range(4):
        for c in range(ndma):
            xt_img[:, (g * ndma + c) * 512:(g * ndma + c + 1) * 512] = \
                xT_aug[c * P:(c + 1) * P, g * 512:(g + 1) * 512]
    return {
        "xT": xt_img.astype(BF16),
        "wq": wq_img.astype(BF16),
        "wk": wk_img.astype(BF16),
        "wv": wv_img.astype(BF16),
        "wo": wo_img.astype(BF16),
    }


def kernel(normalized_resid_pre, W_Q, W_K, W_V, W_O, b_Q, b_K, b_V, b_O):
    x = np.asarray(normalized_resid_pre, np.float32)
    mc = 6 if not (np.any(b_Q) or np.any(b_K) or np.any(b_V)) else 7
    nc = _get_nc(mc)

    in_maps = []
    for core in range(NCORES):
        b, t = divmod(core, 2)
        hs = slice(t * HPC, (t + 1) * HPC)
        in_maps.append(_prep_core_inputs(
            x[b], np.asarray(W_Q)[hs], np.asarray(W_K)[hs], np.asarray(W_V)[hs],
            np.asarray(W_O)[hs], np.asarray(b_Q)[hs], np.asarray(b_K)[hs],
            np.asarray(b_V)[hs], mc=mc,
        ))

    res = run_bass_kernel_spmd(nc, in_maps, core_ids=list(range(NCORES)))
    out = np.zeros((4, S, DM), np.float32)
    for b in range(4):
        out[b] = (res.results[2 * b]["out"].astype(np.float32)
                  + res.results[2 * b + 1]["out"].astype(np.float32))
        out[b] += np.asarray(b_O, np.float32)
    return out
